# revision 1
# baseline (speedup 1.0000x reference)
"""Trainium2 Bass kernel for nn_CLIPVisionTower (latent-token attention block).

Strategy: data-parallel over batch (16 batches -> 8 cores x 2). Each core runs the
full block for its 2 batch elements; host concatenates outputs. No collectives.

Layout notes:
- All weights are passed host-transposed ([d_in, d_out]) and declared as float32r
  DRAM (raw fp32 bytes; the PE f32r path runs 1 cycle/row at free-dim >= 256).
- Activations flow "transposed" (feature dim on SBUF partitions): kvT holds
  [latt | hidden]^T per batch. Keys are zero-padded to 768 rows so every
  attention s-chunk is a full 128-row matmul; padded keys contribute exp(0)=1
  against V rows that are exactly zero and a ones-column that the host zeroes,
  so no masking instructions are needed.
- Softmax skips max-subtraction (logits*0.125 is O(5); exp is safe in fp32) and
  runs in key-major layout: exp on ScalarE straight out of PSUM; the Z row rides
  along as a 65th ones-column of V; 1/Z is computed by DVE reciprocal straight
  from PSUM and broadcast across partitions with a partition-stride-0 SWDGE DMA.
- Attention tensors (Q^T, K^T, V, exp(probs)) are bf16; projections are f32r.
"""

import sys

sys.path.insert(0, "/opt/trn_rl_repo")

import numpy as np
import ml_dtypes

import concourse.bass as bass
import concourse.mybir as mybir
import concourse.tile as tile
from concourse import bacc
from concourse.bass_utils import run_bass_kernel_spmd
from concourse.masks import make_identity

B, T, D = 16, 577, 1024
L, D_LLM = 64, 4096
H, HD = 16, 64
SCALE = HD ** -0.5
S = L + T            # 641 kv rows
SP = 768             # padded key rows for attention (6 * 128)
NC = 8               # cores
BPC = B // NC        # batches per core = 2

F32 = mybir.dt.float32
F32R = mybir.dt.float32r
BF16 = mybir.dt.bfloat16
Exp = mybir.ActivationFunctionType.Exp
Identity = mybir.ActivationFunctionType.Identity
MULT = mybir.AluOpType.mult

# f32r matmuls need even output widths
TQ = 578             # padded token axis for QT
SQ = 642             # padded kv column count (even)
TC2 = [(0, 290), (290, 578)]          # Q-proj psum chunks
KC_HI = (384, 642)                    # K-proj s-chunk independent of latt
KC_LO = (0, 384)                      # K-proj s-chunk that needs latt


def build_nc(zero_bias: bool):
    nc = bacc.Bacc(None, target_bir_lowering=False)

    kvT_d = nc.dram_tensor("kvT", [BPC, D, TQ], F32R, kind="ExternalInput")
    lrT_d = nc.dram_tensor("lrT", [D_LLM, BPC * L], BF16, kind="ExternalInput")
    WpT_d = nc.dram_tensor("WpT", [D_LLM, D], BF16, kind="ExternalInput")
    WqT_d = nc.dram_tensor("WqT", [D, D], F32R, kind="ExternalInput")
    WkT_d = nc.dram_tensor("WkT", [D, D], F32R, kind="ExternalInput")
    WvT_d = nc.dram_tensor("WvT", [D, D], F32R, kind="ExternalInput")
    WoT_d = nc.dram_tensor("WoT", [D, D], F32R, kind="ExternalInput")
    bq_d = nc.dram_tensor("bq2", [128, 8], F32, kind="ExternalInput")
    bk_d = nc.dram_tensor("bk2", [128, 8], F32, kind="ExternalInput")
    bp_d = nc.dram_tensor("bp2", [128, 8], F32, kind="ExternalInput")
    bv_d = nc.dram_tensor("bv2", [128, 8], F32, kind="ExternalInput")
    bo_d = nc.dram_tensor("bo2", [1, D], F32R, kind="ExternalInput")
    onesv_d = nc.dram_tensor("onesv", [128, 6, BPC, H], BF16, kind="ExternalInput")
    out_d = nc.dram_tensor("out2", [BPC, T, D], F32, kind="ExternalOutput")

    with tile.TileContext(nc) as tc:
        with tc.tile_pool(name="big", bufs=1) as big:
            # ---- persistent tensors ----
            QT = big.tile([128, 8, BPC, TQ], BF16, tag="qt")
            KT = big.tile([128, 8, BPC, SP], BF16, tag="kt")
            V = big.tile([128, 6, BPC, H * 65], BF16, tag="v")
            ctxT = big.tile([128, 8, BPC, T], F32R, tag="ctx")

            ident = big.tile([128, 128], BF16, tag="ident")
            bq_sb = big.tile([128, 8], F32, tag="bq")
            bk_sb = big.tile([128, 8], F32, tag="bk")
            bp_sb = big.tile([128, 8], F32, tag="bp")
            bv_sb = big.tile([128, 8], F32, tag="bv")

            with (
                tc.tile_pool(name="kvpool", bufs=1) as kvpool,
                tc.tile_pool(name="wpool", bufs=2) as wpool,
                tc.tile_pool(name="pp", bufs=3, space="PSUM") as pp,
            ):
                kv_sb = kvpool.tile([128, 8, BPC, SQ], F32R, tag="kv")
                kv_re = kvT_d[:, :, :].rearrange("b (j p) s -> p j b s", p=128)
                wq_re = WqT_d[:, :].rearrange("(k p) o -> p k o", p=128)
                wk_re = WkT_d[:, :].rearrange("(k p) o -> p k o", p=128)
                wv_re = WvT_d[:, :].rearrange("(k p) o -> p k o", p=128)
                wp_re = WpT_d[:, :].rearrange("(k p) o -> p k o", p=128)

                # ---- Q projection (emitted first: smallest DMA prefix) ----
                wq = [None, None]
                for oh in range(2):
                    wq[oh] = wpool.tile([128, 8, 512], F32R, tag="w", name=f"wq{oh}")
                    nc.sync.dma_start(wq[oh], wq_re[:, :, oh * 512:(oh + 1) * 512])
                    if oh == 0:
                        for b in range(BPC):
                            nc.sync.dma_start(
                                kv_sb[:, :, b, L:SQ], kv_re[:, :, b, :])
                        # consts ride behind the critical loads
                        make_identity(nc, ident)
                        nc.sync.dma_start(bq_sb, bq_d[:, :])
                        nc.sync.dma_start(bk_sb, bk_d[:, :])
                        nc.sync.dma_start(bp_sb, bp_d[:, :])
                        nc.sync.dma_start(bv_sb, bv_d[:, :])
                    for b in range(BPC):
                        for j4 in range(4):
                            j = oh * 4 + j4
                            for (t0, t1) in TC2:
                                w = t1 - t0
                                ps = pp.tile([128, 512], F32, tag="pp")
                                for k in range(8):
                                    nc.tensor.matmul(
                                        ps[:, :w],
                                        wq[oh][:, k, j4 * 128:(j4 + 1) * 128],
                                        kv_sb[:, k, b, L + t0:L + t1],
                                        start=(k == 0), stop=(k == 7),
                                    )
                                nc.scalar.activation(
                                    QT[:, j, b, t0:t1], ps[:, :w],
                                    Identity, bias=bq_sb[:, j:j + 1],
                                )

                # ---- K projection part 1: latt-independent s-chunk ----
                wk = [None, None]
                nc.vector.memset(KT[:, :, :, S:SP], 0.0)
                for oh in range(2):
                    wk[oh] = wpool.tile([128, 8, 512], F32R, tag="w", name=f"wk{oh}")
                    nc.sync.dma_start(wk[oh], wk_re[:, :, oh * 512:(oh + 1) * 512])

                def k_chunk(oh, s0, s1):
                    w = s1 - s0
                    for b in range(BPC):
                        for j4 in range(4):
                            j = oh * 4 + j4
                            ps = pp.tile([128, 512], F32, tag="pp", name="psk")
                            for k in range(8):
                                nc.tensor.matmul(
                                    ps[:, :w],
                                    wk[oh][:, k, j4 * 128:(j4 + 1) * 128],
                                    kv_sb[:, k, b, s0:s1],
                                    start=(k == 0), stop=(k == 7),
                                )
                            nc.scalar.activation(
                                KT[:, j, b, s0:s1], ps[:, :w],
                                Identity, bias=bk_sb[:, j:j + 1],
                            )

                k_chunk(0, *KC_HI)
                k_chunk(1, *KC_HI)

                # ---- latt = latt_raw @ Wp.T (+bp) into kv columns 0:64 ----
                with tc.tile_pool(name="lat", bufs=1) as lat:
                    lr_sb = lat.tile([128, 32, BPC * L], BF16, tag="lr")
                    nc.sync.dma_start(
                        lr_sb, lrT_d[:, :].rearrange("(k p) l -> p k l", p=128)
                    )
                    latn = lat.tile([128, D], BF16, tag="latn")
                    for oc in range(4):
                        ps = pp.tile([128, 512], F32, tag="pp")
                        for kh in range(2):
                            wp = wpool.tile([128, 16, 256], BF16, tag="wp")
                            nc.sync.dma_start(
                                wp, wp_re[:, kh * 16:(kh + 1) * 16,
                                          oc * 256:(oc + 1) * 256]
                            )
                            for k in range(16):
                                nc.tensor.matmul(
                                    ps[:, :256],
                                    lr_sb[:, kh * 16 + k, :],
                                    wp[:, k, :],
                                    start=(kh == 0 and k == 0),
                                    stop=(kh == 1 and k == 15),
                                )
                        nc.vector.tensor_copy(
                            latn[:, oc * 256:(oc + 1) * 256], ps[:, :256]
                        )
                    for j in range(8):
                        pt = pp.tile([128, 128], BF16, tag="ptr")
                        nc.tensor.transpose(
                            pt, latn[:, j * 128:(j + 1) * 128], ident
                        )
                        for b in range(BPC):
                            nc.vector.tensor_scalar_add(
                                kv_sb[:, j, b, 0:L],
                                pt[:, b * L:(b + 1) * L],
                                bp_sb[:, j:j + 1],
                            )

                # ---- K projection part 2: chunk that needs latt ----
                k_chunk(0, *KC_LO)
                k_chunk(1, *KC_LO)

                # ---- V projection: natural [s, o], 65-stride head blocks ----
                nc.vector.memset(V[:, 5, :, :], 0.0)
                wv = [None, None]
                for oh in range(2):
                    wv[oh] = wpool.tile([128, 8, 512], F32R, tag="w", name=f"wv{oh}")
                    nc.sync.dma_start(wv[oh], wv_re[:, :, oh * 512:(oh + 1) * 512])
                for oh in range(2):
                    for b in range(BPC):
                        for sc in (1, 2, 3, 4, 0, 5):
                            m = 128 if sc < 5 else 1
                            ps = pp.tile([128, 512], F32, tag="pp")
                            for k in range(8):
                                nc.tensor.matmul(
                                    ps[:m, :],
                                    kv_sb[:, k, b, sc * 128:sc * 128 + m],
                                    wv[oh][:, k, :],
                                    start=(k == 0), stop=(k == 7),
                                )
                            vv = V[:, sc, b, :].rearrange("p (h c) -> p h c", c=65)
                            nc.vector.tensor_copy(
                                vv[:m, oh * 8:(oh + 1) * 8, 0:64],
                                ps[:m, :].rearrange("p (h c) -> p h c", c=64),
                            )
                vv_all = V[:, :, :, :].rearrange("p s b (h c) -> p s b h c", c=65)
                nc.sync.dma_start(vv_all[:, :, :, :, 64], onesv_d[:, :, :, :])

            # ---- attention ----
            with (
                tc.tile_pool(name="att", bufs=1) as att,
                tc.tile_pool(name="expp", bufs=4) as expp,
                tc.tile_pool(name="zp", bufs=3) as zp,
                tc.tile_pool(name="zdp", bufs=6, space="DRAM") as zdp,
                tc.tile_pool(name="osb", bufs=3) as osbp,
            ):
                wo = att.tile([128, 8, D], F32R, tag="wo")
                nc.sync.dma_start(wo, WoT_d[:, :].rearrange("(k p) o -> p k o", p=128))
                if not zero_bias:
                    ones1_f = att.tile([1, 128], F32, tag="ones1f")
                    nc.vector.memset(ones1_f, 1.0)
                    ones1 = att.tile([1, 128], F32R, tag="ones1")
                    nc.vector.tensor_copy(ones1, ones1_f)
                    bo_sb = att.tile([1, D], F32R, tag="bo")
                    nc.sync.dma_start(bo_sb, bo_d[:, :])

                import contextlib
                _stk = contextlib.ExitStack()
                ppa = _stk.enter_context(tc.tile_pool(name="pa", bufs=2, space="PSUM"))
                ppb = _stk.enter_context(tc.tile_pool(name="pb", bufs=2, space="PSUM"))
                ppv = _stk.enter_context(tc.tile_pool(name="pv", bufs=2, space="PSUM"))

                for jp in range(8):
                    for b in range(BPC):
                        # head pair (2jp, 2jp+1): even head on PE rows 0-63,
                        # odd head on rows 64-127 -> adjacent matmuls overlap
                        ea2, pbv2, pv02, pv12 = [], [], [], []
                        for g in range(3):
                            pa2 = []
                            for hh in range(2):
                                hb = 64 * hh
                                if g == 0:
                                    ea2.append(expp.tile(
                                        [128, 6, T], BF16, tag="ea",
                                        name=f"ea{hh}"))
                                    pbv = ppb.tile([128, 7, 65], F32,
                                                   tag="pbv", name=f"pbv{hh}")
                                    pbv2.append(pbv)
                                pa2.append(ppa.tile(
                                    [128, 2, 512], F32, tag="pa",
                                    name=f"pa{hh}"))
                            for sc2 in range(2):
                                sc = g * 2 + sc2
                                for hh in range(2):
                                    hb = 64 * hh
                                    kt = KT[hb:hb + 64, jp, b,
                                            sc * 128:(sc + 1) * 128]
                                    nc.tensor.matmul(
                                        pa2[hh][:, sc2, :], kt,
                                        QT[hb:hb + 64, jp, b, 0:512],
                                        start=True, stop=True,
                                    )
                                for hh in range(2):
                                    hb = 64 * hh
                                    kt = KT[hb:hb + 64, jp, b,
                                            sc * 128:(sc + 1) * 128]
                                    nc.tensor.matmul(
                                        pbv2[hh][:, sc, :], kt,
                                        QT[hb:hb + 64, jp, b, 512:T],
                                        start=True, stop=True,
                                    )
                            for hh in range(2):
                                nc.scalar.activation(
                                    ea2[hh][:, g * 2:(g + 1) * 2, 0:512],
                                    pa2[hh], Exp, bias=0.0, scale=SCALE,
                                )
                        for hh in range(2):
                            nc.scalar.activation(
                                ea2[hh][:, :, 512:T], pbv2[hh][:, 0:6, :],
                                Exp, bias=0.0, scale=SCALE,
                            )

                        for hh in range(2):
                            h, hb = 2 * jp + hh, 64 * hh
                            j = jp
                            ea = ea2[hh]
                            pv0 = ppv.tile([65, 512], F32, tag="pv0",
                                           name=f"pv0{hh}")
                            pv1 = pbv2[hh][0:65, 6, :]
                            for sc in range(6):
                                vh = V[:, sc, b, 65 * h:65 * h + 65]
                                nc.tensor.matmul(pv0, vh, ea[:, sc, 0:512],
                                                 start=(sc == 0), stop=(sc == 5))
                                nc.tensor.matmul(pv1, vh, ea[:, sc, 512:T],
                                                 start=(sc == 0), stop=(sc == 5))

                            # evacuate PV psum fast; broadcast 1/Z across
                            # partitions via DRAM-bounce stride-0 DMA
                            zs = zp.tile([65, T], F32, tag="zs")
                            nc.vector.reciprocal(zs[64:65, 0:512], pv0[64:65, :])
                            nc.vector.reciprocal(zs[64:65, 512:T], pv1[64:65, :])
                            pvs = zp.tile([64, T], F32, tag="pvs")
                            nc.vector.tensor_copy(pvs[:, 0:512], pv0[0:64, :])
                            nc.vector.tensor_copy(pvs[:, 512:T], pv1[0:64, :])
                            zb = zp.tile([64, T], F32, tag="zb")
                            zd = zdp.tile([1, T], F32, tag="zd")
                            nc.gpsimd.dma_start(zd, zs[64:65, :])
                            zdsrc = zd[0:1, :]
                            src = bass.AP(
                                tensor=zdsrc.tensor, offset=zdsrc.offset,
                                ap=[[0, 64]] + [list(d) for d in zdsrc.ap[1:]],
                            )
                            nc.gpsimd.dma_start(zb, src)
                            nc.vector.tensor_tensor(
                                ctxT[hb:hb + 64, j, b, 0:512],
                                pvs[:, 0:512], zb[:, 0:512], MULT,
                            )
                            nc.vector.tensor_tensor(
                                ctxT[hb:hb + 64, j, b, 512:T],
                                pvs[:, 512:T], zb[:, 512:T], MULT,
                            )
                            if not zero_bias:
                                for (t0, t1) in ((0, 512), (512, T)):
                                    nc.vector.tensor_scalar_add(
                                        ctxT[hb:hb + 64, j, b, t0:t1],
                                        ctxT[hb:hb + 64, j, b, t0:t1],
                                        bv_sb[hb:hb + 64, j:j + 1],
                                    )

                _stk.close()

                # ---- output projection: out[t, o] = ctxT.T @ WoT (+bo) ----
                import contextlib as _ctxlib
                _stk2 = _ctxlib.ExitStack()
                ppo = _stk2.enter_context(
                    tc.tile_pool(name="ppo", bufs=4, space="PSUM"))
                for b in range(BPC):
                    for tcn in range(5):
                        t0 = tcn * 128
                        m = min(128, T - t0)
                        osb = osbp.tile([128, D], F32, tag="osb")
                        for oc in range(2):
                            ps = ppo.tile([128, 512], F32, tag="ppo")
                            for k in range(8):
                                nc.tensor.matmul(
                                    ps[:m, :],
                                    ctxT[:, k, b, t0:t0 + m],
                                    wo[:, k, oc * 512:(oc + 1) * 512],
                                    start=(k == 0), stop=(zero_bias and k == 7),
                                )
                            if not zero_bias:
                                nc.tensor.matmul(
                                    ps[:m, :], ones1[0:1, :m],
                                    bo_sb[0:1, oc * 512:(oc + 1) * 512],
                                    start=False, stop=True,
                                )
                            nc.scalar.copy(
                                osb[:m, oc * 512:(oc + 1) * 512], ps[:m, :]
                            )
                        nc.sync.dma_start(out_d[b, t0:t0 + m, :], osb[:m, :])
                _stk2.close()

    nc.finalize()
    return nc


_NC_CACHE = {}
LAST_RESULT = None


def kernel(hidden_states, latt_raw, Wp, bp, Wq, bq, Wk, bk, Wv, bv, Wo, bo,
           trace=False):
    global LAST_RESULT
    f = lambda x: np.ascontiguousarray(np.asarray(x), dtype=np.float32)
    hs, lr = f(hidden_states), f(latt_raw)
    Wp, Wq, Wk, Wv, Wo = f(Wp), f(Wq), f(Wk), f(Wv), f(Wo)
    bp, bq, bk, bv, bo = f(bp), f(bq), f(bk), f(bv), f(bo)

    zero_bias = not any(x.any() for x in (bp, bq, bk, bv, bo))
    key = zero_bias
    if key not in _NC_CACHE:
        _NC_CACHE[key] = build_nc(zero_bias)
    nc = _NC_CACHE[key]

    WpT = np.ascontiguousarray(Wp.T.astype(ml_dtypes.bfloat16))
    WqT = np.ascontiguousarray(Wq.T)
    WkT = np.ascontiguousarray(Wk.T)
    WvT = np.ascontiguousarray(Wv.T)
    WoT = np.ascontiguousarray(Wo.T)
    b2 = lambda x: np.ascontiguousarray(x.reshape(8, 128).T)
    bq2, bk2, bp2, bv2 = b2(bq), b2(bk), b2(bp), b2(bv)
    bo2 = np.ascontiguousarray(bo[None, :])

    p = np.arange(128)[:, None]
    sc = np.arange(6)[None, :]
    valid = (sc * 128 + p) < S                       # [128, 6]
    onesv = np.broadcast_to(
        valid[:, :, None, None], (128, 6, BPC, H)
    ).astype(ml_dtypes.bfloat16)
    onesv = np.ascontiguousarray(onesv)

    in_maps = []
    for c in range(NC):
        hsb = hs[c * BPC:(c + 1) * BPC]              # [2, 577, 1024]
        kvt = np.zeros((BPC, D, TQ), np.float32)
        kvt[:, :, 0:T] = hsb.transpose(0, 2, 1)
        lrt = np.concatenate(
            [lr[c * BPC + b].T for b in range(BPC)], axis=1
        ).astype(ml_dtypes.bfloat16)                  # [4096, 128]
        in_maps.append({
            "kvT": kvt, "lrT": np.ascontiguousarray(lrt),
            "WpT": WpT, "WqT": WqT, "WkT": WkT, "WvT": WvT, "WoT": WoT,
            "bq2": bq2, "bk2": bk2, "bp2": bp2, "bv2": bv2, "bo2": bo2,
            "onesv": onesv,
        })

    LAST_RESULT = run_bass_kernel_spmd(
        nc, in_maps, core_ids=list(range(NC)), trace=trace
    )
    outs = [r["out2"] for r in LAST_RESULT.results]
    return np.ascontiguousarray(np.concatenate(outs, axis=0), dtype=np.float32)



# revision 25
# speedup vs baseline: 1.1590x; 1.1590x over previous
"""Trainium2 Bass kernel for nn_CLIPVisionTower (latent-token attention block).

Strategy: data-parallel over batch (16 batches -> 8 cores x 2). Each core runs
the full block for its 2 batch elements; host concatenates outputs.

Design notes (cost model: matmul cost = out-free-size x cycles/row, K/M free):
- Everything bf16 (weights, activations, attention tensors); psum f32.
- kv^T layout [d on partitions, s free]: cols 0:64 latents (computed on
  device), 64:641 hidden tokens. S = 641 = 5*128 + 1: five full 128-row
  s-chunks + ONE straggler row (s=640), handled without a padded 6th chunk:
  * V row 640 via transposed-orientation matmuls (N=2 instead of N=512).
  * K col 640 packed block-diagonally into kstragT [128, 8, 2, 16] so ONE
    matmul chain per batch yields all 16 heads' straggler logits [16, 577].
  * Straggler probs are slot-scattered (via a DRAM bounce) to partition
    base 32*b so the K=1 rank-1 PV update satisfies tile_position rules.
- bk is never added: softmax is invariant to a per-query logit shift.
- Softmax skips max-subtraction; Z rides as a 65th ones-column of V; ctx is
  normalized by a DVE divide against a Z row broadcast across partitions
  with a DRAM-bounce stride-0 DMA (one bounce per head pair).
- Out-projection of batch 0 is interleaved into batch 1's attention loop.
"""

import sys

sys.path.insert(0, "/opt/trn_rl_repo")

import numpy as np
import ml_dtypes

import concourse.bass as bass
import concourse.mybir as mybir
import concourse.tile as tile
from concourse import bacc
from concourse.bass_utils import run_bass_kernel_spmd
from concourse.masks import make_identity

B, T, D = 16, 577, 1024
L, D_LLM = 64, 4096
H, HD = 16, 64
SCALE = HD ** -0.5
S = L + T            # 641 kv rows
NC = 8               # cores
BPC = B // NC        # batches per core = 2

F32 = mybir.dt.float32
BF16 = mybir.dt.bfloat16
Exp = mybir.ActivationFunctionType.Exp
Identity = mybir.ActivationFunctionType.Identity
MULT = mybir.AluOpType.mult


def _ap(base, offset_delta, dims):
    """Hand-built AP: keep base's tensor/partition dim, custom free dims."""
    return bass.AP(
        tensor=base.tensor,
        offset=base.offset + offset_delta,
        ap=[list(base.ap[0])] + [list(d) for d in dims],
    )


def build_nc(zero_bias: bool, debug: bool = False):
    nc = bacc.Bacc(None, target_bir_lowering=False)

    hskv_d = nc.dram_tensor("hskv", [128, 8, BPC, T], BF16, kind="ExternalInput")
    lrT_d = nc.dram_tensor("lrT", [128, 32, BPC * L], BF16, kind="ExternalInput")
    WqTj_d = nc.dram_tensor("WqTj", [128, 8, 8, 128], BF16, kind="ExternalInput")
    WkT_d = nc.dram_tensor("WkT", [128, 8, D], BF16, kind="ExternalInput")
    WvT_d = nc.dram_tensor("WvT", [128, 8, D], BF16, kind="ExternalInput")
    WoT_d = nc.dram_tensor("WoT", [128, 8, D], BF16, kind="ExternalInput")
    WpT_d = nc.dram_tensor("WpT", [128, 32, D], BF16, kind="ExternalInput")
    if not zero_bias:
        bq_d = nc.dram_tensor("bq2", [128, 8], F32, kind="ExternalInput")
        bp_d = nc.dram_tensor("bp2", [128, 8], F32, kind="ExternalInput")
        bv_d = nc.dram_tensor("bv2", [128, 8], F32, kind="ExternalInput")
        bo_d = nc.dram_tensor("bo2", [1, D], BF16, kind="ExternalInput")
    out_d = nc.dram_tensor("out2", [BPC, T, D], F32, kind="ExternalOutput")
    if debug:
        dbg = {
            "dQT": nc.dram_tensor("dQT", [128, 8, BPC, T], BF16,
                                  kind="ExternalOutput"),
            "dKT": nc.dram_tensor("dKT", [128, 8, BPC, S], BF16,
                                  kind="ExternalOutput"),
            "dV": nc.dram_tensor("dV", [128, 5, BPC, H * 65], BF16,
                                 kind="ExternalOutput"),
            "dkv": nc.dram_tensor("dkv", [128, 8, BPC, S], BF16,
                                  kind="ExternalOutput"),
            "des4": nc.dram_tensor("des4", [64, 16, 640], BF16,
                                   kind="ExternalOutput"),
            "dvstrag": nc.dram_tensor("dvstrag", [64, 16, 65], BF16,
                                      kind="ExternalOutput"),
            "dkstragT": nc.dram_tensor("dkstragT", [128, 8, BPC, 16], BF16,
                                       kind="ExternalOutput"),
            "dctxT": nc.dram_tensor("dctxT", [128, 8, BPC, T], BF16,
                                    kind="ExternalOutput"),
        }

    with tile.TileContext(nc) as tc:
        with (
            tc.tile_pool(name="big", bufs=1) as big,
            tc.tile_pool(name="esp", bufs=2) as esp,
            tc.tile_pool(name="drp", bufs=4, space="DRAM") as drp,
        ):
            QT = big.tile([128, 8, BPC, T], BF16, tag="qt")
            KT = big.tile([128, 8, BPC, S], BF16, tag="kt")
            V = big.tile([128, 5, BPC, H * 65], BF16, tag="v")
            ctxT = big.tile([128, 8, BPC, T], BF16, tag="ctx")
            vstrag = big.tile([64, 16, 65], BF16, tag="vstrag")
            kstragT = big.tile([128, 8, BPC, 16], BF16, tag="kstragT")
            es4 = big.tile([64, 16, 640], BF16, tag="es4")
            if not zero_bias:
                bq_sb = big.tile([128, 8], F32, tag="bq")
                bp_sb = big.tile([128, 8], F32, tag="bp")
                bv_sb = big.tile([128, 8], F32, tag="bv")
                bo_sb = big.tile([1, D], BF16, tag="bo")
                ones1 = big.tile([1, 128], BF16, tag="ones1")

            def evac(dst, src, scalar_eng, bias=None):
                if bias is not None:
                    nc.scalar.activation(dst, src, Identity, bias=bias)
                elif scalar_eng:
                    nc.scalar.copy(dst, src)
                else:
                    nc.vector.tensor_copy(dst, src)

            with (
                tc.tile_pool(name="kvpool", bufs=1) as kvpool,
                tc.tile_pool(name="wpool", bufs=2) as wpool,
                tc.tile_pool(name="wppool", bufs=2) as wppool,
                tc.tile_pool(name="pp", bufs=2, space="PSUM") as pp,
                tc.tile_pool(name="latp_p", bufs=1, space="PSUM") as latp_p,
            ):
                kv = kvpool.tile([128, 8, BPC, S], BF16, tag="kv")
                if debug:
                    nc.vector.memset(vstrag, 0.0)
                    nc.vector.memset(es4, 0.0)
                    nc.vector.memset(kv, 0.0)
                lr_sb = kvpool.tile([128, 32, BPC * L], BF16, tag="lr")

                # ---- DMA kickoff: few chunky DMAs (HWDGE issue ~0.6us) ----
                wq = wpool.tile([128, 8, 8, 128], BF16, tag="w", name="wq")
                nc.sync.dma_start(wq[:, 0], WqTj_d[:, 0])
                for k in range(0, 8, 4):
                    nc.sync.dma_start(kv[:, k:k + 4, 0, L:S],
                                      hskv_d[:, k:k + 4, 0, :])
                nc.sync.dma_start(wq[:, 1:4], WqTj_d[:, 1:4])
                for k in range(0, 8, 4):
                    nc.sync.dma_start(kv[:, k:k + 4, 1, L:S],
                                      hskv_d[:, k:k + 4, 1, :])
                nc.sync.dma_start(wq[:, 4:8], WqTj_d[:, 4:8])
                wk = wpool.tile([128, 8, D], BF16, tag="w", name="wk")
                for i in range(2):
                    nc.sync.dma_start(wk[:, 4 * i:4 * i + 4, :],
                                      WkT_d[:, 4 * i:4 * i + 4, :])
                nc.sync.dma_start(lr_sb, lrT_d[:, :, :])
                if not zero_bias:
                    nc.sync.dma_start(bq_sb, bq_d[:, :])
                    nc.sync.dma_start(bp_sb, bp_d[:, :])
                    nc.sync.dma_start(bv_sb, bv_d[:, :])
                    nc.sync.dma_start(bo_sb, bo_d[:, :])
                    nc.vector.memset(ones1, 1.0)

                # ---- Q projection: QT[o, t] ----
                for b in range(BPC):
                    for j in range(8):
                        ps = pp.tile([128, 2, 512], F32, tag="pq")
                        for k in range(8):
                            nc.tensor.matmul(
                                ps[:, 0, :], wq[:, j, k, :],
                                kv[:, k, b, L:L + 512],
                                start=(k == 0), stop=(k == 7))
                        for k in range(8):
                            nc.tensor.matmul(
                                ps[:, 1, 0:65], wq[:, j, k, :],
                                kv[:, k, b, L + 512:S],
                                start=(k == 0), stop=(k == 7))
                        bias = None if zero_bias else bq_sb[:, j:j + 1]
                        evac(QT[:, j, b, 0:512], ps[:, 0, :], True, bias)
                        evac(QT[:, j, b, 512:T], ps[:, 1, 0:65], False, bias)

                # wv reuses wq's buffer (Q-proj consumed it above)
                wv = wpool.tile([128, 8, D], BF16, tag="w", name="wv")
                for i in range(4):
                    nc.sync.dma_start(wv[:, 2 * i:2 * i + 2, :],
                                      WvT_d[:, 2 * i:2 * i + 2, :])

                def k_hi(b):
                    """K projection, tokens part (s = 64..641); no bk ever."""
                    for j in range(8):
                        ps = pp.tile([128, 2, 512], F32, tag="pq", name="psk")
                        for k in range(8):
                            nc.tensor.matmul(
                                ps[:, 0, :], wk[:, k, j * 128:(j + 1) * 128],
                                kv[:, k, b, L:L + 512],
                                start=(k == 0), stop=(k == 7))
                        for k in range(8):
                            nc.tensor.matmul(
                                ps[:, 1, 0:65], wk[:, k, j * 128:(j + 1) * 128],
                                kv[:, k, b, L + 512:S],
                                start=(k == 0), stop=(k == 7))
                        evac(KT[:, j, b, L:L + 512], ps[:, 0, :], True)
                        evac(KT[:, j, b, L + 512:S], ps[:, 1, 0:65], False)

                k_hi(0)

                # ---- straggler row s=640 (token 576): K and V, transposed ----
                ps = pp.tile([128, 2, 512], F32, tag="pq", name="ps8")
                s8p = ps[:, 0, 0:32].rearrange("p (o c) -> p o c", c=4)
                for oc in range(8):
                    for k in range(8):
                        nc.tensor.matmul(
                            s8p[:, oc, 0:2], wk[:, k, oc * 128:(oc + 1) * 128],
                            kv[:, k, 0:BPC, S - 1],
                            start=(k == 0), stop=(k == 7))
                    for k in range(8):
                        nc.tensor.matmul(
                            s8p[:, oc, 2:4], wv[:, k, oc * 128:(oc + 1) * 128],
                            kv[:, k, 0:BPC, S - 1],
                            start=(k == 0), stop=(k == 7))
                s8 = kvpool.tile([128, 8, 4], BF16, tag="s8")
                nc.vector.tensor_copy(s8, s8p)

                # kstragT[p, k, b, h] = K[128k+p, 640] iff head(128k+p)==h
                nc.vector.memset(kstragT, 0.0)
                s8f = s8.rearrange("p o c -> p (o c)")
                kf = kstragT.rearrange("p k b h -> p (k b h)")
                for half in range(2):
                    pr = slice(64 * half, 64 * half + 64)
                    dst = _ap(kf[pr, 0:1], half, [[34, 8], [16, BPC]])
                    src = _ap(s8f[pr, 0:1], 0, [[4, 8], [1, BPC]])
                    nc.vector.tensor_copy(dst, src)

                # vstrag slot-scatter via DRAM bounce: slot (h, b) lives at
                # partition 32*b, free slot h
                d8 = drp.tile([128, 8, 4], BF16, tag="d8")
                nc.gpsimd.dma_start(d8, s8)
                nc.vector.memset(vstrag[:, :, 64:65], 1.0)
                for b in range(BPC):
                    for h in range(16):
                        par, oc = h % 2, h // 2
                        dst = vstrag[32 * b:32 * b + 1, h, 0:64]
                        src = _ap(d8[0:1, 0:1, 0],
                                  64 * par * 32 + oc * 4 + 2 + b, [[32, 64]])
                        nc.gpsimd.dma_start(dst, src)

                def strag_logits(b):
                    """All 16 heads' straggler logits [16, 577] -> es4."""
                    pls = pp.tile([128, 2, 512], F32, tag="pq", name="pls")
                    for k in range(8):
                        nc.tensor.matmul(
                            pls[0:16, 0, :], kstragT[:, k, b, :],
                            QT[:, k, b, 0:512],
                            start=(k == 0), stop=(k == 7))
                    for k in range(8):
                        nc.tensor.matmul(
                            pls[0:16, 1, 0:65], kstragT[:, k, b, :],
                            QT[:, k, b, 512:T],
                            start=(k == 0), stop=(k == 7))
                    es_sb = esp.tile([16, 640], BF16, tag="es")
                    nc.vector.memset(es_sb[:, T:640], 0.0)
                    nc.scalar.activation(es_sb[:, 0:512], pls[0:16, 0, :],
                                         Exp, bias=0.0, scale=SCALE)
                    nc.scalar.activation(es_sb[:, 512:T], pls[0:16, 1, 0:65],
                                         Exp, bias=0.0, scale=SCALE)
                    esd = drp.tile([16, 640], BF16, tag="esd")
                    nc.gpsimd.dma_start(esd, es_sb)
                    dst = es4[32 * b:32 * b + 1, :, :].rearrange(
                        "p f c -> p (f c)")
                    src = _ap(esd[0:1, 0:1], 0, [[1, 16 * 640]])
                    nc.gpsimd.dma_start(dst, src)

                strag_logits(0)
                k_hi(1)

                # ---- V proj main (s-chunks 1..4) interleaved with latt ----
                # latt in [lat, o] orientation: one accumulation group per
                # psum bank (a start= claims a whole 2KB zero region)
                latp = latp_p.tile([128, 2, 512], F32, tag="lat")
                ident = kvpool.tile([128, 128], BF16, tag="ident")
                make_identity(nc, ident)

                def v_main(b, scs):
                    for sc in scs:
                        ps = pp.tile([128, 2, 512], F32, tag="pq", name="psv")
                        for oh in range(2):
                            for k in range(8):
                                nc.tensor.matmul(
                                    ps[:, oh, :],
                                    kv[:, k, b, sc * 128:(sc + 1) * 128],
                                    wv[:, k, oh * 512:(oh + 1) * 512],
                                    start=(k == 0), stop=(k == 7))
                        vv = V[:, sc, b, :].rearrange("p (h c) -> p h c", c=65)
                        for oh in range(2):
                            evac(vv[:, oh * 8:(oh + 1) * 8, 0:64],
                                 ps[:, oh, :].rearrange(
                                     "p (h c) -> p h c", c=64),
                                 oh == 0)

                def latt_q(hq):
                    """One eighth (4 k-chunks) of the latt contraction."""
                    wp = wppool.tile([128, 4, D], BF16, tag="wp")
                    nc.sync.dma_start(wp, WpT_d[:, hq * 4:(hq + 1) * 4, :])
                    for k in range(4):
                        for oh in range(2):
                            nc.tensor.matmul(
                                latp[:, oh, :],
                                lr_sb[:, hq * 4 + k, :],
                                wp[:, k, oh * 512:(oh + 1) * 512],
                                start=(hq == 0 and k == 0),
                                stop=(hq == 7 and k == 3))

                v_main(0, (1, 2))
                latt_q(0)
                latt_q(1)
                v_main(0, (3, 4))
                latt_q(2)
                latt_q(3)
                v_main(1, (1, 2))
                latt_q(4)
                latt_q(5)
                v_main(1, (3, 4))
                latt_q(6)
                latt_q(7)
                latn = kvpool.tile([128, D], BF16, tag="latn")
                evac(latn[:, 0:512], latp[:, 0, :], True)
                evac(latn[:, 512:D], latp[:, 1, :], False)
                for oc in range(8):
                    ptb = latp_p.tile([128, 128], BF16, tag="ptb")
                    nc.tensor.transpose(
                        ptb, latn[:, oc * 128:(oc + 1) * 128], ident)
                    bias = None if zero_bias else bp_sb[:, oc:oc + 1]
                    evac(kv[:, oc, :, 0:L],
                         ptb[:, :].rearrange("p (b l) -> p b l", b=BPC),
                         oc % 2 == 0, bias)

                # ---- K proj latents part (s 0..64) + V chunk 0, mixed ----
                def k_lo(j):
                    ps = pp.tile([128, 2, 512], F32, tag="pq", name="pkl")
                    for b in range(BPC):
                        for k in range(8):
                            nc.tensor.matmul(
                                ps[:, b, 0:L],
                                wk[:, k, j * 128:(j + 1) * 128],
                                kv[:, k, b, 0:L],
                                start=(k == 0), stop=(k == 7))
                    evac(KT[:, j, :, 0:L],
                         ps[:, 0:2, 0:L], j % 2 == 0)

                def v_0(b):
                    ps = pp.tile([128, 2, 512], F32, tag="pq", name="psv0")
                    for oh in range(2):
                        for k in range(8):
                            nc.tensor.matmul(
                                ps[:, oh, :], kv[:, k, b, 0:128],
                                wv[:, k, oh * 512:(oh + 1) * 512],
                                start=(k == 0), stop=(k == 7))
                    vv = V[:, 0, b, :].rearrange("p (h c) -> p h c", c=65)
                    for oh in range(2):
                        evac(vv[:, oh * 8:(oh + 1) * 8, 0:64],
                             ps[:, oh, :].rearrange("p (h c) -> p h c", c=64),
                             oh == 0)

                k_lo(0)
                k_lo(1)
                v_0(0)
                k_lo(2)
                k_lo(3)
                k_lo(4)
                v_0(1)
                k_lo(5)
                k_lo(6)
                k_lo(7)
                if debug:
                    nc.sync.dma_start(dbg["dkv"][:, :, :, :], kv)
                # ones column of V (all rows 0..640 are valid keys)
                vva = V[:, :, :, :].rearrange("p s b (h c) -> p s b h c", c=65)
                nc.vector.memset(vva[:, :, :, :, 64:65], 1.0)

            # ---- attention + out-projection (PV software-pipelined) ----
            with (
                tc.tile_pool(name="att", bufs=1) as att,
                tc.tile_pool(name="eap", bufs=12) as eap,
                tc.tile_pool(name="ebp", bufs=3) as ebp,
                tc.tile_pool(name="pvsp", bufs=3) as pvsp,
                tc.tile_pool(name="zbp", bufs=3) as zbp,
                tc.tile_pool(name="zrp", bufs=3) as zrp,
                tc.tile_pool(name="osbp", bufs=3) as osbp,
                tc.tile_pool(name="pa", bufs=4, space="PSUM") as pa_p,
            ):
                wo = att.tile([128, 8, D], BF16, tag="wo")
                for i in range(2):
                    nc.sync.dma_start(wo[:, 4 * i:4 * i + 4, :],
                                      WoT_d[:, 4 * i:4 * i + 4, :])

                def out_proj(b, tcn, scalar_evac):
                    t0 = tcn * 128
                    m = min(128, T - t0)
                    po = pa_p.tile([128, 2, 512], F32, tag="pa", name="po")
                    osb = osbp.tile([128, D], F32, tag="osb")
                    for oc in range(2):
                        for k in range(8):
                            nc.tensor.matmul(
                                po[:m, oc, :], ctxT[:, k, b, t0:t0 + m],
                                wo[:, k, oc * 512:(oc + 1) * 512],
                                start=(k == 0), stop=(zero_bias and k == 7))
                        if not zero_bias:
                            nc.tensor.matmul(
                                po[:m, oc, :], ones1[0:1, :m],
                                bo_sb[0:1, oc * 512:(oc + 1) * 512],
                                start=False, stop=True)
                        evac(osb[:m, oc * 512:(oc + 1) * 512], po[:m, oc, :],
                             scalar_evac)
                        nc.sync.dma_start(
                            out_d[b, t0:t0 + m, oc * 512:(oc + 1) * 512],
                            osb[:m, oc * 512:(oc + 1) * 512])

                def lg(jp, b, ebv, ea, g):
                    """One logits s-chunk (t 0:512 part) + its exp."""
                    pa = pa_p.tile([128, 2, 512], F32, tag="pa")
                    for hh in range(2):
                        hb = 64 * hh
                        kt = KT[hb:hb + 64, jp, b, g * 128:(g + 1) * 128]
                        nc.tensor.matmul(
                            pa[:, hh, :], kt, QT[hb:hb + 64, jp, b, 0:512],
                            start=True, stop=True)
                    eg = eap.tile([128, 2, 512], BF16, tag="ea")
                    nc.scalar.activation(eg, pa, Exp, bias=0.0, scale=SCALE)
                    ea.append(eg)

                def eb_mms(jp, b, ebv):
                    """All 65-col logits matmuls (t 512:577) + one exp."""
                    for g in range(5):
                        for hh in range(2):
                            hb = 64 * hh
                            kt = KT[hb:hb + 64, jp, b, g * 128:(g + 1) * 128]
                            nc.tensor.matmul(
                                ebv[1][hh][:, g, :], kt,
                                QT[hb:hb + 64, jp, b, 512:T],
                                start=True, stop=True)
                    ebs = ebp.tile([128, 2, 5, 65], BF16, tag="eb")
                    nc.scalar.activation(
                        ebs, ebv[0][:, :, 0:325].rearrange(
                            "p h (g c) -> p h g c", c=65),
                        Exp, bias=0.0, scale=SCALE)
                    return ebs

                def pv_mm(st, sc):
                    (jp, b, ea, ebs, pvt) = st
                    hh = sc // 6
                    h = 2 * jp + hh
                    pv0 = pvt[hh][0:65, 0, :]
                    pv1 = pvt[hh][0:65, 1, 0:65]
                    scc = sc % 6
                    if scc < 5:
                        vh = V[:, scc, b, 65 * h:65 * h + 65]
                        nc.tensor.matmul(pv0, vh, ea[scc][:, hh, :],
                                         start=(scc == 0), stop=False)
                        nc.tensor.matmul(pv1, vh, ebs[:, hh, scc, :],
                                         start=(scc == 0), stop=False)
                    else:
                        nc.tensor.matmul(
                            pv0, vstrag[32 * b:32 * b + 1, h, :],
                            es4[32 * b:32 * b + 1, h, 0:512],
                            start=False, stop=True)
                        nc.tensor.matmul(
                            pv1, vstrag[32 * b:32 * b + 1, h, :],
                            es4[32 * b:32 * b + 1, h, 512:T],
                            start=False, stop=True)

                def pv_evac(st, pvs, zr, hh):
                    pvt = st[4]
                    nc.vector.tensor_copy(pvs[0:64, hh, 0:512],
                                          pvt[hh][0:64, 0, :])
                    nc.vector.tensor_copy(pvs[0:64, hh, 512:T],
                                          pvt[hh][0:64, 1, 0:65])
                    nc.vector.reciprocal(zr[:, hh, 0:512],
                                         pvt[hh][64:65, 0, :])
                    nc.vector.reciprocal(zr[:, hh, 512:T],
                                         pvt[hh][64:65, 1, 0:65])

                def z_bounce(st, pvs, zr):
                    (jp, b) = st[0], st[1]
                    zd = drp.tile([1, 2, T], F32, tag="zd")
                    nc.sync.dma_start(zd, zr[0:1, :, 0:T])
                    zb = zbp.tile([64, 2, T], F32, tag="zb")
                    for hh in range(2):
                        zsrc = zd[0:1, hh, :]
                        src = bass.AP(
                            tensor=zsrc.tensor, offset=zsrc.offset,
                            ap=[[0, 64]] + [list(d) for d in zsrc.ap[1:]])
                        nc.gpsimd.dma_start(zb[:, hh, :], src)

                    def divides():
                        for hh in range(2):
                            hb = 64 * hh
                            for (t0, t1) in ((0, 512), (512, T)):
                                nc.vector.tensor_tensor(
                                    ctxT[hb:hb + 64, jp, b, t0:t1],
                                    pvs[0:64, hh, t0:t1],
                                    zb[:, hh, t0:t1], MULT)
                                if not zero_bias:
                                    nc.vector.tensor_scalar_add(
                                        ctxT[hb:hb + 64, jp, b, t0:t1],
                                        ctxT[hb:hb + 64, jp, b, t0:t1],
                                        bv_sb[hb:hb + 64, jp:jp + 1])
                    return divides

                def strag_logits2():
                    """b=1 straggler logits, emitted piecewise in window 0."""
                    pls = pa_p.tile([128, 2, 512], F32, tag="pa", name="pls")

                    def p1():
                        for k in range(8):
                            nc.tensor.matmul(
                                pls[0:16, 0, :], kstragT[:, k, 1, :],
                                QT[:, k, 1, 0:512],
                                start=(k == 0), stop=(k == 7))

                    def p2():
                        for k in range(8):
                            nc.tensor.matmul(
                                pls[0:16, 1, 0:65], kstragT[:, k, 1, :],
                                QT[:, k, 1, 512:T],
                                start=(k == 0), stop=(k == 7))

                    def p3():
                        es_sb = esp.tile([16, 640], BF16, tag="es")
                        nc.vector.memset(es_sb[:, T:640], 0.0)
                        nc.scalar.activation(es_sb[:, 0:512], pls[0:16, 0, :],
                                             Exp, bias=0.0, scale=SCALE)
                        nc.scalar.activation(es_sb[:, 512:T],
                                             pls[0:16, 1, 0:65],
                                             Exp, bias=0.0, scale=SCALE)
                        esd = drp.tile([16, 640], BF16, tag="esd")
                        nc.gpsimd.dma_start(esd, es_sb)
                        dst = es4[32:33, :, :].rearrange("p f c -> p (f c)")
                        srcx = _ap(esd[0:1, 0:1], 0, [[1, 16 * 640]])
                        nc.gpsimd.dma_start(dst, srcx)
                    return (p1, p2, p3)

                iters = [(jp, b) for b in range(BPC) for jp in range(8)]
                prev = None      # (jp, b, ea, ebs, pvt) awaiting PV
                pending = []     # divide closures awaiting flush
                sl2 = None
                for it in iters + [None]:
                    if len(pending) >= 2:
                        pending.pop(0)()
                    if prev is not None:
                        pvs = pvsp.tile([64, 2, 578], F32, tag="pvs")
                        zr = zrp.tile([1, 2, 578], F32, tag="zr")
                    if it is not None:
                        (jp, b) = it
                        ea = []
                        ebt = pa_p.tile([128, 2, 512], F32, tag="pa",
                                        name="ebt")
                        ebv = (ebt, [ebt[:, hh, 0:325].rearrange(
                            "p (g c) -> p g c", c=65) for hh in range(2)])
                        if prev is None:
                            sl2 = strag_logits2()
                        lg(jp, b, ebv, ea, 0)
                        if prev is not None:
                            prev[4].append(pa_p.tile(
                                [128, 2, 512], F32, tag="pa", name="pvt0"))
                            pv_mm(prev, 0)
                            pv_mm(prev, 1)
                        elif sl2 is not None:
                            sl2[0]()
                        lg(jp, b, ebv, ea, 1)
                        lg(jp, b, ebv, ea, 2)
                        if prev is not None:
                            pv_mm(prev, 2)
                            pv_mm(prev, 3)
                        elif sl2 is not None:
                            sl2[1]()
                        lg(jp, b, ebv, ea, 3)
                        if prev is not None:
                            pv_mm(prev, 4)
                            pv_mm(prev, 5)
                            pv_evac(prev, pvs, zr, 0)
                            prev[4].append(pa_p.tile(
                                [128, 2, 512], F32, tag="pa", name="pvt1"))
                            for sc in (6, 7, 8):
                                pv_mm(prev, sc)
                        lg(jp, b, ebv, ea, 4)
                        if prev is not None:
                            for sc in (9, 10, 11):
                                pv_mm(prev, sc)
                            pv_evac(prev, pvs, zr, 1)
                            pending.append(z_bounce(prev, pvs, zr))
                        elif sl2 is not None:
                            sl2[2]()
                            sl2 = None
                        ebs = eb_mms(jp, b, ebv)
                        if b == 1 and 2 <= jp <= 6:
                            out_proj(0, jp - 2, False)
                        prev = (jp, b, ea, ebs, [])
                    else:
                        # drain: PV of the last iteration
                        prev[4].append(pa_p.tile(
                            [128, 2, 512], F32, tag="pa", name="pvt0"))
                        for sc in range(6):
                            pv_mm(prev, sc)
                        pv_evac(prev, pvs, zr, 0)
                        prev[4].append(pa_p.tile(
                            [128, 2, 512], F32, tag="pa", name="pvt1"))
                        for sc in range(6, 12):
                            pv_mm(prev, sc)
                        pv_evac(prev, pvs, zr, 1)
                        for p_ in pending:
                            p_()
                        z_bounce(prev, pvs, zr)()

                for tcn in range(5):
                    out_proj(1, tcn, True)
                if debug:
                    nc.sync.dma_start(dbg["dQT"][:, :, :], QT)
                    nc.sync.dma_start(dbg["dKT"][:, :, :], KT)
                    nc.sync.dma_start(dbg["dV"][:, :, :], V)
                    nc.sync.dma_start(dbg["des4"][:, :, :], es4)
                    nc.sync.dma_start(dbg["dvstrag"][:, :, :], vstrag)
                    nc.sync.dma_start(dbg["dkstragT"][:, :, :], kstragT)
                    nc.sync.dma_start(dbg["dctxT"][:, :, :], ctxT)

    nc.finalize()
    return nc


_NC_CACHE = {}
LAST_RESULT = None


def kernel(hidden_states, latt_raw, Wp, bp, Wq, bq, Wk, bk, Wv, bv, Wo, bo,
           trace=False):
    global LAST_RESULT
    f = lambda x: np.ascontiguousarray(np.asarray(x), dtype=np.float32)
    hs, lr = f(hidden_states), f(latt_raw)
    Wp, Wq, Wk, Wv, Wo = f(Wp), f(Wq), f(Wk), f(Wv), f(Wo)
    bp, bq, bk, bv, bo = f(bp), f(bq), f(bk), f(bv), f(bo)

    # bk never matters: softmax is invariant to the per-query shift q . bk
    zero_bias = not any(x.any() for x in (bp, bq, bv, bo))
    key = zero_bias
    if key not in _NC_CACHE:
        _NC_CACHE[key] = build_nc(zero_bias)
    nc = _NC_CACHE[key]

    bf = ml_dtypes.bfloat16
    # [j, p, k, m] = Wq[128j+m, 128k+p]
    WqTj = np.ascontiguousarray(
        Wq.reshape(8, 128, 8, 128).transpose(3, 0, 2, 1).astype(bf))
    # [p, k, o] = W[o, 128k+p]
    t8 = lambda W: np.ascontiguousarray(
        W.T.reshape(8, 128, D).transpose(1, 0, 2).astype(bf))
    WkT, WvT, WoT = t8(Wk), t8(Wv), t8(Wo)
    WpT = np.ascontiguousarray(
        Wp.T.reshape(32, 128, D).transpose(1, 0, 2).astype(bf))

    b2 = lambda x: np.ascontiguousarray(x.reshape(8, 128).T)
    bias_maps = {}
    if not zero_bias:
        bias_maps = {
            "bq2": b2(bq), "bp2": b2(bp), "bv2": b2(bv),
            "bo2": np.ascontiguousarray(bo[None, :].astype(bf)),
        }

    in_maps = []
    for c in range(NC):
        hsb = hs[c * BPC:(c + 1) * BPC]              # [2, 577, 1024]
        hskv = np.ascontiguousarray(
            hsb.reshape(BPC, T, 8, 128).transpose(3, 2, 0, 1).astype(bf))
        lrb = lr[c * BPC:(c + 1) * BPC]              # [2, 64, 4096]
        lrT = np.ascontiguousarray(
            lrb.reshape(BPC, L, 32, 128).transpose(3, 2, 0, 1)
            .reshape(128, 32, BPC * L).astype(bf))
        m = {"hskv": hskv, "lrT": lrT, "WqTj": WqTj, "WkT": WkT,
             "WvT": WvT, "WoT": WoT, "WpT": WpT}
        m.update(bias_maps)
        in_maps.append(m)

    LAST_RESULT = run_bass_kernel_spmd(
        nc, in_maps, core_ids=list(range(NC)), trace=trace
    )
    outs = [r["out2"] for r in LAST_RESULT.results]
    return np.ascontiguousarray(np.concatenate(outs, axis=0), dtype=np.float32)


# revision 43
# speedup vs baseline: 1.1824x; 1.0203x over previous
"""Trainium2 Bass kernel for nn_CLIPVisionTower (latent-token attention block).

Strategy: data-parallel over batch (16 batches -> 8 cores x 2). Each core runs
the full block for its 2 batch elements; host concatenates outputs.

Design notes (cost model: matmul cost = out-free-size x cycles/row, K/M free):
- Everything bf16 (weights, activations, attention tensors); psum f32.
- kv^T layout [d on partitions, s free]: cols 0:64 latents (computed on
  device), 64:641 hidden tokens. S = 641 = 5*128 + 1: five full 128-row
  s-chunks + ONE straggler row (s=640), handled without a padded 6th chunk:
  * V row 640 via transposed-orientation matmuls (N=2 instead of N=512).
  * K col 640 packed block-diagonally into kstragT [128, 8, 2, 16] so ONE
    matmul chain per batch yields all 16 heads' straggler logits [16, 577].
  * Straggler probs are slot-scattered (via a DRAM bounce) to partition
    base 32*b so the K=1 rank-1 PV update satisfies tile_position rules.
- bk is never added: softmax is invariant to a per-query logit shift.
- Softmax skips max-subtraction; Z rides as a 65th ones-column of V; ctx is
  normalized by a DVE divide against a Z row broadcast across partitions
  with a DRAM-bounce stride-0 DMA (one bounce per head pair).
- Out-projection of batch 0 is interleaved into batch 1's attention loop.
"""

import sys

sys.path.insert(0, "/opt/trn_rl_repo")

import numpy as np
import ml_dtypes

import concourse.bass as bass
import concourse.mybir as mybir
import concourse.tile as tile
from concourse import bacc
from concourse.bass_utils import run_bass_kernel_spmd
from concourse.masks import make_identity

B, T, D = 16, 577, 1024
L, D_LLM = 64, 4096
H, HD = 16, 64
SCALE = HD ** -0.5
S = L + T            # 641 kv rows
NC = 8               # cores
BPC = B // NC        # batches per core = 2

F32 = mybir.dt.float32
BF16 = mybir.dt.bfloat16
Exp = mybir.ActivationFunctionType.Exp
Identity = mybir.ActivationFunctionType.Identity
MULT = mybir.AluOpType.mult


def _ap(base, offset_delta, dims):
    """Hand-built AP: keep base's tensor/partition dim, custom free dims."""
    return bass.AP(
        tensor=base.tensor,
        offset=base.offset + offset_delta,
        ap=[list(base.ap[0])] + [list(d) for d in dims],
    )


def build_nc(zero_bias: bool, debug: bool = False):
    nc = bacc.Bacc(None, target_bir_lowering=False)

    hskv_d = nc.dram_tensor("hskv", [128, 8, BPC, T], BF16, kind="ExternalInput")
    lrT_d = nc.dram_tensor("lrT", [128, 32, BPC * L], BF16, kind="ExternalInput")
    WqTj_d = nc.dram_tensor("WqTj", [128, 8, 8, 128], BF16, kind="ExternalInput")
    WkT_d = nc.dram_tensor("WkT", [128, 8, D], BF16, kind="ExternalInput")
    WvT_d = nc.dram_tensor("WvT", [128, 8, D], BF16, kind="ExternalInput")
    WoT_d = nc.dram_tensor("WoT", [128, 8, D], BF16, kind="ExternalInput")
    WpT_d = nc.dram_tensor("WpT", [128, 32, D], BF16, kind="ExternalInput")
    if not zero_bias:
        bq_d = nc.dram_tensor("bq2", [128, 8], F32, kind="ExternalInput")
        bp_d = nc.dram_tensor("bp2", [128, 8], F32, kind="ExternalInput")
        bv_d = nc.dram_tensor("bv2", [128, 8], F32, kind="ExternalInput")
        bo_d = nc.dram_tensor("bo2", [1, D], BF16, kind="ExternalInput")
    out_d = nc.dram_tensor("out2", [BPC, T, D], BF16, kind="ExternalOutput")
    if debug:
        dbg = {
            "dQT": nc.dram_tensor("dQT", [128, 8, BPC, T], BF16,
                                  kind="ExternalOutput"),
            "dKT": nc.dram_tensor("dKT", [128, 8, BPC, S], BF16,
                                  kind="ExternalOutput"),
            "dV": nc.dram_tensor("dV", [128, 5, BPC, H * 65], BF16,
                                 kind="ExternalOutput"),
            "dkv": nc.dram_tensor("dkv", [128, 8, BPC, S], BF16,
                                  kind="ExternalOutput"),
            "des4": nc.dram_tensor("des4", [64, 16, 640], BF16,
                                   kind="ExternalOutput"),
            "dvstrag": nc.dram_tensor("dvstrag", [64, 16, 65], BF16,
                                      kind="ExternalOutput"),
            "dkstragT": nc.dram_tensor("dkstragT", [128, 8, BPC, 16], BF16,
                                       kind="ExternalOutput"),
            "dctxT": nc.dram_tensor("dctxT", [128, 8, BPC, T], BF16,
                                    kind="ExternalOutput"),
        }

    with tile.TileContext(nc) as tc:
        with (
            tc.tile_pool(name="big", bufs=1) as big,
            tc.tile_pool(name="esp", bufs=2) as esp,
            tc.tile_pool(name="drp", bufs=4, space="DRAM") as drp,
        ):
            QT = big.tile([128, 8, BPC, T], BF16, tag="qt")
            KT = big.tile([128, 8, BPC, S], BF16, tag="kt")
            V = big.tile([128, 5, BPC, H * 65], BF16, tag="v")
            ctxT = big.tile([128, 8, BPC, T], BF16, tag="ctx")
            vstrag = big.tile([64, 16, 65], BF16, tag="vstrag")
            kstragT = big.tile([128, 8, BPC, 16], BF16, tag="kstragT")
            es4 = big.tile([64, 16, 640], BF16, tag="es4")
            if not zero_bias:
                bq_sb = big.tile([128, 8], F32, tag="bq")
                bp_sb = big.tile([128, 8], F32, tag="bp")
                bv_sb = big.tile([128, 8], F32, tag="bv")
                bo_sb = big.tile([1, D], BF16, tag="bo")
                ones1 = big.tile([1, 128], BF16, tag="ones1")

            def evac(dst, src, scalar_eng, bias=None):
                if bias is not None:
                    nc.scalar.activation(dst, src, Identity, bias=bias)
                elif scalar_eng:
                    nc.scalar.copy(dst, src)
                else:
                    nc.vector.tensor_copy(dst, src)

            with (
                tc.tile_pool(name="kvpool", bufs=1) as kvpool,
                tc.tile_pool(name="wpool", bufs=2) as wpool,
                tc.tile_pool(name="wppool", bufs=2) as wppool,
                tc.tile_pool(name="pp", bufs=2, space="PSUM") as pp,
                tc.tile_pool(name="latp_p", bufs=1, space="PSUM") as latp_p,
            ):
                kv = kvpool.tile([128, 8, BPC, S], BF16, tag="kv")
                if debug:
                    nc.vector.memset(vstrag, 0.0)
                    nc.vector.memset(es4, 0.0)
                    nc.vector.memset(kv, 0.0)
                lr_sb = kvpool.tile([128, 32, BPC * L], BF16, tag="lr")

                # ---- DMA kickoff: few chunky DMAs (HWDGE issue ~0.6us) ----
                wq = wpool.tile([128, 8, 8, 128], BF16, tag="w", name="wq")
                nc.sync.dma_start(wq[:, 0], WqTj_d[:, 0])
                nc.sync.dma_start(kv[:, 0:2, 0, L:S], hskv_d[:, 0:2, 0, :])
                nc.sync.dma_start(kv[:, 2:8, 0, L:S], hskv_d[:, 2:8, 0, :])
                nc.sync.dma_start(wq[:, 1:4], WqTj_d[:, 1:4])
                for k in range(0, 8, 4):
                    nc.sync.dma_start(kv[:, k:k + 4, 1, L:S],
                                      hskv_d[:, k:k + 4, 1, :])
                nc.sync.dma_start(wq[:, 4:8], WqTj_d[:, 4:8])
                wk = wpool.tile([128, 8, D], BF16, tag="w", name="wk")
                for i in range(2):
                    nc.sync.dma_start(wk[:, 4 * i:4 * i + 4, :],
                                      WkT_d[:, 4 * i:4 * i + 4, :])
                nc.sync.dma_start(lr_sb, lrT_d[:, :, :])
                if not zero_bias:
                    nc.sync.dma_start(bq_sb, bq_d[:, :])
                    nc.sync.dma_start(bp_sb, bp_d[:, :])
                    nc.sync.dma_start(bv_sb, bv_d[:, :])
                    nc.sync.dma_start(bo_sb, bo_d[:, :])
                    nc.vector.memset(ones1, 1.0)

                # ---- Q projection: QT[o, t] ----
                for b in range(BPC):
                    for j in range(8):
                        ps = pp.tile([128, 2, 512], F32, tag="pq")
                        for k in range(8):
                            nc.tensor.matmul(
                                ps[:, 0, :], wq[:, j, k, :],
                                kv[:, k, b, L:L + 512],
                                start=(k == 0), stop=(k == 7))
                        for k in range(8):
                            nc.tensor.matmul(
                                ps[:, 1, 0:65], wq[:, j, k, :],
                                kv[:, k, b, L + 512:S],
                                start=(k == 0), stop=(k == 7))
                        bias = None if zero_bias else bq_sb[:, j:j + 1]
                        evac(QT[:, j, b, 0:512], ps[:, 0, :], True, bias)
                        evac(QT[:, j, b, 512:T], ps[:, 1, 0:65], False, bias)

                # wv reuses wq's buffer (Q-proj consumed it above)
                wv = wpool.tile([128, 8, D], BF16, tag="w", name="wv")
                for i in range(4):
                    nc.sync.dma_start(wv[:, 2 * i:2 * i + 2, :],
                                      WvT_d[:, 2 * i:2 * i + 2, :])

                def k_hi(b):
                    """K projection, tokens part (s = 64..641); no bk ever."""
                    for j in range(8):
                        ps = pp.tile([128, 2, 512], F32, tag="pq", name="psk")
                        for k in range(8):
                            nc.tensor.matmul(
                                ps[:, 0, :], wk[:, k, j * 128:(j + 1) * 128],
                                kv[:, k, b, L:L + 512],
                                start=(k == 0), stop=(k == 7))
                        for k in range(8):
                            nc.tensor.matmul(
                                ps[:, 1, 0:65], wk[:, k, j * 128:(j + 1) * 128],
                                kv[:, k, b, L + 512:S],
                                start=(k == 0), stop=(k == 7))
                        evac(KT[:, j, b, L:L + 512], ps[:, 0, :], True)
                        evac(KT[:, j, b, L + 512:S], ps[:, 1, 0:65], False)

                k_hi(0)

                # ---- straggler row s=640 (token 576): K and V, transposed ----
                ps = pp.tile([128, 2, 512], F32, tag="pq", name="ps8")
                s8p = ps[:, 0, 0:32].rearrange("p (o c) -> p o c", c=4)
                for oc in range(8):
                    for k in range(8):
                        nc.tensor.matmul(
                            s8p[:, oc, 0:2], wk[:, k, oc * 128:(oc + 1) * 128],
                            kv[:, k, 0:BPC, S - 1],
                            start=(k == 0), stop=(k == 7))
                    for k in range(8):
                        nc.tensor.matmul(
                            s8p[:, oc, 2:4], wv[:, k, oc * 128:(oc + 1) * 128],
                            kv[:, k, 0:BPC, S - 1],
                            start=(k == 0), stop=(k == 7))
                s8 = kvpool.tile([128, 8, 4], BF16, tag="s8")
                nc.vector.tensor_copy(s8, s8p)

                # kstragT[p, k, b, h] = K[128k+p, 640] iff head(128k+p)==h
                nc.vector.memset(kstragT, 0.0)
                s8f = s8.rearrange("p o c -> p (o c)")
                kf = kstragT.rearrange("p k b h -> p (k b h)")
                for half in range(2):
                    pr = slice(64 * half, 64 * half + 64)
                    dst = _ap(kf[pr, 0:1], half, [[34, 8], [16, BPC]])
                    src = _ap(s8f[pr, 0:1], 0, [[4, 8], [1, BPC]])
                    nc.vector.tensor_copy(dst, src)

                # vstrag slot-scatter via DRAM bounce: slot (h, b) lives at
                # partition 32*b, free slot h
                d8 = drp.tile([128, 8, 4], BF16, tag="d8")
                nc.gpsimd.dma_start(d8, s8)
                nc.vector.memset(vstrag[:, :, 64:65], 1.0)
                for b in range(BPC):
                    for h in range(16):
                        par, oc = h % 2, h // 2
                        dst = vstrag[32 * b:32 * b + 1, h, 0:64]
                        src = _ap(d8[0:1, 0:1, 0],
                                  64 * par * 32 + oc * 4 + 2 + b, [[32, 64]])
                        nc.gpsimd.dma_start(dst, src)

                def strag_logits(b):
                    """All 16 heads' straggler logits [16, 577] -> es4."""
                    pls = pp.tile([128, 2, 512], F32, tag="pq", name="pls")
                    for k in range(8):
                        nc.tensor.matmul(
                            pls[0:16, 0, :], kstragT[:, k, b, :],
                            QT[:, k, b, 0:512],
                            start=(k == 0), stop=(k == 7))
                    for k in range(8):
                        nc.tensor.matmul(
                            pls[0:16, 1, 0:65], kstragT[:, k, b, :],
                            QT[:, k, b, 512:T],
                            start=(k == 0), stop=(k == 7))
                    es_sb = esp.tile([16, 640], BF16, tag="es")
                    nc.vector.memset(es_sb[:, T:640], 0.0)
                    nc.scalar.activation(es_sb[:, 0:512], pls[0:16, 0, :],
                                         Exp, bias=0.0, scale=SCALE)
                    nc.scalar.activation(es_sb[:, 512:T], pls[0:16, 1, 0:65],
                                         Exp, bias=0.0, scale=SCALE)
                    esd = drp.tile([16, 640], BF16, tag="esd")
                    nc.gpsimd.dma_start(esd, es_sb)
                    dst = es4[32 * b:32 * b + 1, :, :].rearrange(
                        "p f c -> p (f c)")
                    src = _ap(esd[0:1, 0:1], 0, [[1, 16 * 640]])
                    nc.gpsimd.dma_start(dst, src)

                strag_logits(0)
                k_hi(1)

                # ---- V proj main (s-chunks 1..4) interleaved with latt ----
                # latt in [lat, o] orientation: one accumulation group per
                # psum bank (a start= claims a whole 2KB zero region)
                latp = latp_p.tile([128, 2, 512], F32, tag="lat")
                ident = kvpool.tile([128, 128], BF16, tag="ident")
                make_identity(nc, ident)

                def v_main(b, scs):
                    for sc in scs:
                        ps = pp.tile([128, 2, 512], F32, tag="pq", name="psv")
                        for oh in range(2):
                            for k in range(8):
                                nc.tensor.matmul(
                                    ps[:, oh, :],
                                    kv[:, k, b, sc * 128:(sc + 1) * 128],
                                    wv[:, k, oh * 512:(oh + 1) * 512],
                                    start=(k == 0), stop=(k == 7))
                        vv = V[:, sc, b, :].rearrange("p (h c) -> p h c", c=65)
                        for oh in range(2):
                            evac(vv[:, oh * 8:(oh + 1) * 8, 0:64],
                                 ps[:, oh, :].rearrange(
                                     "p (h c) -> p h c", c=64),
                                 oh == 0)

                def latt_q(hq):
                    """One eighth (4 k-chunks) of the latt contraction."""
                    wp = wppool.tile([128, 4, D], BF16, tag="wp")
                    nc.sync.dma_start(wp, WpT_d[:, hq * 4:(hq + 1) * 4, :])
                    for k in range(4):
                        for oh in range(2):
                            nc.tensor.matmul(
                                latp[:, oh, :],
                                lr_sb[:, hq * 4 + k, :],
                                wp[:, k, oh * 512:(oh + 1) * 512],
                                start=(hq == 0 and k == 0),
                                stop=(hq == 7 and k == 3))

                v_main(0, (1, 2))
                latt_q(0)
                latt_q(1)
                v_main(0, (3, 4))
                latt_q(2)
                latt_q(3)
                v_main(1, (1, 2))
                latt_q(4)
                latt_q(5)
                v_main(1, (3, 4))
                latt_q(6)
                latt_q(7)
                latn = kvpool.tile([128, D], BF16, tag="latn")
                evac(latn[:, 0:512], latp[:, 0, :], True)
                evac(latn[:, 512:D], latp[:, 1, :], False)
                for oc in range(8):
                    ptb = latp_p.tile([128, 128], BF16, tag="ptb")
                    nc.tensor.transpose(
                        ptb, latn[:, oc * 128:(oc + 1) * 128], ident)
                    bias = None if zero_bias else bp_sb[:, oc:oc + 1]
                    evac(kv[:, oc, :, 0:L],
                         ptb[:, :].rearrange("p (b l) -> p b l", b=BPC),
                         oc % 2 == 0, bias)

                # ---- K proj latents part (s 0..64) + V chunk 0, mixed ----
                def k_lo(j):
                    ps = pp.tile([128, 2, 512], F32, tag="pq", name="pkl")
                    for b in range(BPC):
                        for k in range(8):
                            nc.tensor.matmul(
                                ps[:, b, 0:L],
                                wk[:, k, j * 128:(j + 1) * 128],
                                kv[:, k, b, 0:L],
                                start=(k == 0), stop=(k == 7))
                    evac(KT[:, j, :, 0:L],
                         ps[:, 0:2, 0:L], j % 2 == 0)

                def v_0(b):
                    ps = pp.tile([128, 2, 512], F32, tag="pq", name="psv0")
                    for oh in range(2):
                        for k in range(8):
                            nc.tensor.matmul(
                                ps[:, oh, :], kv[:, k, b, 0:128],
                                wv[:, k, oh * 512:(oh + 1) * 512],
                                start=(k == 0), stop=(k == 7))
                    vv = V[:, 0, b, :].rearrange("p (h c) -> p h c", c=65)
                    for oh in range(2):
                        evac(vv[:, oh * 8:(oh + 1) * 8, 0:64],
                             ps[:, oh, :].rearrange("p (h c) -> p h c", c=64),
                             oh == 0)

                k_lo(0)
                k_lo(1)
                v_0(0)
                k_lo(2)
                k_lo(3)
                k_lo(4)
                v_0(1)
                k_lo(5)
                k_lo(6)
                k_lo(7)
                if debug:
                    nc.sync.dma_start(dbg["dkv"][:, :, :, :], kv)
                # ones column of V (all rows 0..640 are valid keys)
                vva = V[:, :, :, :].rearrange("p s b (h c) -> p s b h c", c=65)
                nc.vector.memset(vva[:, :, :, :, 64:65], 1.0)

            # ---- attention + out-projection (PV software-pipelined) ----
            with (
                tc.tile_pool(name="att", bufs=1) as att,
                tc.tile_pool(name="eap", bufs=12) as eap,
                tc.tile_pool(name="ebp", bufs=3) as ebp,
                tc.tile_pool(name="pvsp", bufs=3) as pvsp,
                tc.tile_pool(name="zbp", bufs=3) as zbp,
                tc.tile_pool(name="zrp", bufs=3) as zrp,
                tc.tile_pool(name="osbp", bufs=3) as osbp,
                tc.tile_pool(name="pa", bufs=4, space="PSUM") as pa_p,
            ):
                wo = att.tile([128, 8, D], BF16, tag="wo")
                for i in range(2):
                    nc.sync.dma_start(wo[:, 4 * i:4 * i + 4, :],
                                      WoT_d[:, 4 * i:4 * i + 4, :])

                def out_proj(b, tcn, scalar_evac):
                    t0 = tcn * 128
                    m = min(128, T - t0)
                    po = pa_p.tile([128, 2, 512], F32, tag="pa", name="po")
                    osb = osbp.tile([128, D], BF16, tag="osb")
                    for oc in range(2):
                        for k in range(8):
                            nc.tensor.matmul(
                                po[:m, oc, :], ctxT[:, k, b, t0:t0 + m],
                                wo[:, k, oc * 512:(oc + 1) * 512],
                                start=(k == 0), stop=(zero_bias and k == 7))
                        if not zero_bias:
                            nc.tensor.matmul(
                                po[:m, oc, :], ones1[0:1, :m],
                                bo_sb[0:1, oc * 512:(oc + 1) * 512],
                                start=False, stop=True)
                        evac(osb[:m, oc * 512:(oc + 1) * 512], po[:m, oc, :],
                             scalar_evac == (oc == 0))
                        nc.sync.dma_start(
                            out_d[b, t0:t0 + m, oc * 512:(oc + 1) * 512],
                            osb[:m, oc * 512:(oc + 1) * 512])

                def lg(jp, b, ebv, ea, g):
                    """One logits s-chunk (t 0:512 part) + its exp."""
                    pa = pa_p.tile([128, 2, 512], F32, tag="pa")
                    for hh in range(2):
                        hb = 64 * hh
                        kt = KT[hb:hb + 64, jp, b, g * 128:(g + 1) * 128]
                        nc.tensor.matmul(
                            pa[:, hh, :], kt, QT[hb:hb + 64, jp, b, 0:512],
                            start=True, stop=True)
                    eg = eap.tile([128, 2, 512], BF16, tag="ea")
                    nc.scalar.activation(eg, pa, Exp, bias=0.0, scale=SCALE)
                    ea.append(eg)

                def eb_mms(jp, b, ebv):
                    """All 65-col logits matmuls (t 512:577) + one exp."""
                    for g in range(5):
                        for hh in range(2):
                            hb = 64 * hh
                            kt = KT[hb:hb + 64, jp, b, g * 128:(g + 1) * 128]
                            nc.tensor.matmul(
                                ebv[1][hh][:, g, :], kt,
                                QT[hb:hb + 64, jp, b, 512:T],
                                start=True, stop=True)
                    ebs = ebp.tile([128, 2, 5, 65], BF16, tag="eb")
                    nc.scalar.activation(
                        ebs, ebv[0][:, :, 0:325].rearrange(
                            "p h (g c) -> p h g c", c=65),
                        Exp, bias=0.0, scale=SCALE)
                    return ebs

                def pv_mm(st, sc):
                    (jp, b, ea, ebs, pvt) = st
                    hh = sc // 6
                    h = 2 * jp + hh
                    pv0 = pvt[hh][0:65, 0, :]
                    pv1 = pvt[hh][0:65, 1, 0:65]
                    scc = sc % 6
                    if scc < 5:
                        vh = V[:, scc, b, 65 * h:65 * h + 65]
                        nc.tensor.matmul(pv0, vh, ea[scc][:, hh, :],
                                         start=(scc == 0), stop=False)
                        nc.tensor.matmul(pv1, vh, ebs[:, hh, scc, :],
                                         start=(scc == 0), stop=False)
                    else:
                        nc.tensor.matmul(
                            pv0, vstrag[32 * b:32 * b + 1, h, :],
                            es4[32 * b:32 * b + 1, h, 0:512],
                            start=False, stop=True)
                        nc.tensor.matmul(
                            pv1, vstrag[32 * b:32 * b + 1, h, :],
                            es4[32 * b:32 * b + 1, h, 512:T],
                            start=False, stop=True)

                def pv_evac(st, pvs, zr, hh):
                    pvf = st[4][hh].rearrange("p a c -> p (a c)")
                    nc.vector.tensor_copy(pvs[0:64, hh, 0:T], pvf[0:64, 0:T])
                    nc.vector.reciprocal(zr[:, hh, 0:T], pvf[64:65, 0:T])

                def z_bounce(st, pvs, zr):
                    (jp, b) = st[0], st[1]
                    zd = drp.tile([1, 2, T], F32, tag="zd")
                    nc.sync.dma_start(zd, zr[0:1, :, 0:T])
                    zb = zbp.tile([64, 2, T], F32, tag="zb")
                    for hh in range(2):
                        zsrc = zd[0:1, hh, :]
                        src = bass.AP(
                            tensor=zsrc.tensor, offset=zsrc.offset,
                            ap=[[0, 64]] + [list(d) for d in zsrc.ap[1:]])
                        nc.gpsimd.dma_start(zb[:, hh, :], src)

                    def divides():
                        for hh in range(2):
                            hb = 64 * hh
                            nc.vector.tensor_tensor(
                                ctxT[hb:hb + 64, jp, b, 0:T],
                                pvs[0:64, hh, 0:T],
                                zb[:, hh, 0:T], MULT)
                            if not zero_bias:
                                nc.vector.tensor_scalar_add(
                                    ctxT[hb:hb + 64, jp, b, 0:T],
                                    ctxT[hb:hb + 64, jp, b, 0:T],
                                    bv_sb[hb:hb + 64, jp:jp + 1])
                    return divides

                def strag_logits2():
                    """b=1 straggler logits, emitted piecewise in window 0."""
                    pls = pa_p.tile([128, 2, 512], F32, tag="pa", name="pls")

                    def p1():
                        for k in range(8):
                            nc.tensor.matmul(
                                pls[0:16, 0, :], kstragT[:, k, 1, :],
                                QT[:, k, 1, 0:512],
                                start=(k == 0), stop=(k == 7))

                    def p2():
                        for k in range(8):
                            nc.tensor.matmul(
                                pls[0:16, 1, 0:65], kstragT[:, k, 1, :],
                                QT[:, k, 1, 512:T],
                                start=(k == 0), stop=(k == 7))

                    def p3():
                        es_sb = esp.tile([16, 640], BF16, tag="es")
                        nc.vector.memset(es_sb[:, T:640], 0.0)
                        nc.scalar.activation(es_sb[:, 0:512], pls[0:16, 0, :],
                                             Exp, bias=0.0, scale=SCALE)
                        nc.scalar.activation(es_sb[:, 512:T],
                                             pls[0:16, 1, 0:65],
                                             Exp, bias=0.0, scale=SCALE)
                        esd = drp.tile([16, 640], BF16, tag="esd")
                        nc.gpsimd.dma_start(esd, es_sb)
                        dst = es4[32:33, :, :].rearrange("p f c -> p (f c)")
                        srcx = _ap(esd[0:1, 0:1], 0, [[1, 16 * 640]])
                        nc.gpsimd.dma_start(dst, srcx)
                    return (p1, p2, p3)

                iters = [(jp, b) for b in range(BPC) for jp in range(8)]
                prev = None      # (jp, b, ea, ebs, pvt) awaiting PV
                pending = []     # divide closures awaiting flush
                sl2 = None
                for it in iters + [None]:
                    if len(pending) >= 2:
                        pending.pop(0)()
                    if prev is not None:
                        pvs = pvsp.tile([64, 2, 578], F32, tag="pvs")
                        zr = zrp.tile([1, 2, 578], F32, tag="zr")
                    if it is not None:
                        (jp, b) = it
                        ea = []
                        ebt = pa_p.tile([128, 2, 512], F32, tag="pa",
                                        name="ebt")
                        ebv = (ebt, [ebt[:, hh, 0:325].rearrange(
                            "p (g c) -> p g c", c=65) for hh in range(2)])
                        if prev is None:
                            sl2 = strag_logits2()
                        lg(jp, b, ebv, ea, 0)
                        if prev is not None:
                            prev[4].append(pa_p.tile(
                                [128, 2, 512], F32, tag="pa", name="pvt0"))
                            pv_mm(prev, 0)
                            pv_mm(prev, 1)
                        elif sl2 is not None:
                            sl2[0]()
                        lg(jp, b, ebv, ea, 1)
                        lg(jp, b, ebv, ea, 2)
                        if prev is not None:
                            pv_mm(prev, 2)
                            pv_mm(prev, 3)
                        elif sl2 is not None:
                            sl2[1]()
                        lg(jp, b, ebv, ea, 3)
                        if prev is not None:
                            pv_mm(prev, 4)
                            pv_mm(prev, 5)
                            pv_evac(prev, pvs, zr, 0)
                            prev[4].append(pa_p.tile(
                                [128, 2, 512], F32, tag="pa", name="pvt1"))
                            for sc in (6, 7, 8):
                                pv_mm(prev, sc)
                        lg(jp, b, ebv, ea, 4)
                        if prev is not None:
                            for sc in (9, 10, 11):
                                pv_mm(prev, sc)
                            pv_evac(prev, pvs, zr, 1)
                            pending.append(z_bounce(prev, pvs, zr))
                        elif sl2 is not None:
                            sl2[2]()
                            sl2 = None
                        ebs = eb_mms(jp, b, ebv)
                        if b == 1 and 2 <= jp <= 6:
                            out_proj(0, jp - 2, False)
                        prev = (jp, b, ea, ebs, [])
                    else:
                        # drain: PV of the last iteration, interleaved with
                        # the k0..k6 accumulation of the first two tail OPs
                        # (their k=7 chunk needs the last divide)
                        for p_ in pending:
                            p_()

                        def op_mms(tcn, po, ks):
                            t0 = tcn * 128
                            m = min(128, T - t0)
                            for oc in range(2):
                                for k in ks:
                                    nc.tensor.matmul(
                                        po[:m, oc, :],
                                        ctxT[:, k, 1, t0:t0 + m],
                                        wo[:, k, oc * 512:(oc + 1) * 512],
                                        start=(k == 0),
                                        stop=(zero_bias and k == 7))

                        def op_fin(tcn, po):
                            t0 = tcn * 128
                            m = min(128, T - t0)
                            if not zero_bias:
                                for oc in range(2):
                                    nc.tensor.matmul(
                                        po[:m, oc, :], ones1[0:1, :m],
                                        bo_sb[0:1, oc * 512:(oc + 1) * 512],
                                        start=False, stop=True)
                            osb = osbp.tile([128, D], BF16, tag="osb")
                            for oc in range(2):
                                evac(osb[:m, oc * 512:(oc + 1) * 512],
                                     po[:m, oc, :], oc == 0)
                                nc.sync.dma_start(
                                    out_d[1, t0:t0 + m,
                                          oc * 512:(oc + 1) * 512],
                                    osb[:m, oc * 512:(oc + 1) * 512])

                        prev[4].append(pa_p.tile(
                            [128, 2, 512], F32, tag="pa", name="pvt0"))
                        po_t0 = pa_p.tile([128, 2, 512], F32, tag="pa",
                                          name="po_t0")
                        op_mms(0, po_t0, range(7))
                        for sc in range(6):
                            pv_mm(prev, sc)
                        pv_evac(prev, pvs, zr, 0)
                        prev[4].append(pa_p.tile(
                            [128, 2, 512], F32, tag="pa", name="pvt1"))
                        po_t1 = pa_p.tile([128, 2, 512], F32, tag="pa",
                                          name="po_t1")
                        op_mms(1, po_t1, range(7))
                        for sc in range(6, 12):
                            pv_mm(prev, sc)
                        pv_evac(prev, pvs, zr, 1)
                        z_bounce(prev, pvs, zr)()
                        op_mms(0, po_t0, range(7, 8))
                        op_fin(0, po_t0)
                        op_mms(1, po_t1, range(7, 8))
                        op_fin(1, po_t1)

                for tcn in range(2, 5):
                    out_proj(1, tcn, True)
                if debug:
                    nc.sync.dma_start(dbg["dQT"][:, :, :], QT)
                    nc.sync.dma_start(dbg["dKT"][:, :, :], KT)
                    nc.sync.dma_start(dbg["dV"][:, :, :], V)
                    nc.sync.dma_start(dbg["des4"][:, :, :], es4)
                    nc.sync.dma_start(dbg["dvstrag"][:, :, :], vstrag)
                    nc.sync.dma_start(dbg["dkstragT"][:, :, :], kstragT)
                    nc.sync.dma_start(dbg["dctxT"][:, :, :], ctxT)

    nc.finalize()
    return nc


_NC_CACHE = {}
LAST_RESULT = None


def kernel(hidden_states, latt_raw, Wp, bp, Wq, bq, Wk, bk, Wv, bv, Wo, bo,
           trace=False):
    global LAST_RESULT
    f = lambda x: np.ascontiguousarray(np.asarray(x), dtype=np.float32)
    hs, lr = f(hidden_states), f(latt_raw)
    Wp, Wq, Wk, Wv, Wo = f(Wp), f(Wq), f(Wk), f(Wv), f(Wo)
    bp, bq, bk, bv, bo = f(bp), f(bq), f(bk), f(bv), f(bo)

    # bk never matters: softmax is invariant to the per-query shift q . bk
    zero_bias = not any(x.any() for x in (bp, bq, bv, bo))
    key = zero_bias
    if key not in _NC_CACHE:
        _NC_CACHE[key] = build_nc(zero_bias)
    nc = _NC_CACHE[key]

    bf = ml_dtypes.bfloat16
    # [j, p, k, m] = Wq[128j+m, 128k+p]
    WqTj = np.ascontiguousarray(
        Wq.reshape(8, 128, 8, 128).transpose(3, 0, 2, 1).astype(bf))
    # [p, k, o] = W[o, 128k+p]
    t8 = lambda W: np.ascontiguousarray(
        W.T.reshape(8, 128, D).transpose(1, 0, 2).astype(bf))
    WkT, WvT, WoT = t8(Wk), t8(Wv), t8(Wo)
    WpT = np.ascontiguousarray(
        Wp.T.reshape(32, 128, D).transpose(1, 0, 2).astype(bf))

    b2 = lambda x: np.ascontiguousarray(x.reshape(8, 128).T)
    bias_maps = {}
    if not zero_bias:
        bias_maps = {
            "bq2": b2(bq), "bp2": b2(bp), "bv2": b2(bv),
            "bo2": np.ascontiguousarray(bo[None, :].astype(bf)),
        }

    in_maps = []
    for c in range(NC):
        hsb = hs[c * BPC:(c + 1) * BPC]              # [2, 577, 1024]
        hskv = np.ascontiguousarray(
            hsb.reshape(BPC, T, 8, 128).transpose(3, 2, 0, 1).astype(bf))
        lrb = lr[c * BPC:(c + 1) * BPC]              # [2, 64, 4096]
        lrT = np.ascontiguousarray(
            lrb.reshape(BPC, L, 32, 128).transpose(3, 2, 0, 1)
            .reshape(128, 32, BPC * L).astype(bf))
        m = {"hskv": hskv, "lrT": lrT, "WqTj": WqTj, "WkT": WkT,
             "WvT": WvT, "WoT": WoT, "WpT": WpT}
        m.update(bias_maps)
        in_maps.append(m)

    LAST_RESULT = run_bass_kernel_spmd(
        nc, in_maps, core_ids=list(range(NC)), trace=trace
    )
    outs = [r["out2"] for r in LAST_RESULT.results]
    return np.ascontiguousarray(np.concatenate(outs, axis=0), dtype=np.float32)


# revision 47
# speedup vs baseline: 1.1912x; 1.0074x over previous
"""Trainium2 Bass kernel for nn_CLIPVisionTower (latent-token attention block).

Strategy: data-parallel over batch (16 batches -> 8 cores x 2). Each core runs
the full block for its 2 batch elements; host concatenates outputs.

Design notes (cost model: matmul cost = out-free-size x cycles/row, K/M free):
- Everything bf16 (weights, activations, attention tensors); psum f32.
- kv^T layout [d on partitions, s free]: cols 0:64 latents (computed on
  device), 64:641 hidden tokens. S = 641 = 5*128 + 1: five full 128-row
  s-chunks + ONE straggler row (s=640), handled without a padded 6th chunk:
  * V row 640 via transposed-orientation matmuls (N=2 instead of N=512).
  * K col 640 packed block-diagonally into kstragT [128, 8, 2, 16] so ONE
    matmul chain per batch yields all 16 heads' straggler logits [16, 577].
  * Straggler probs are slot-scattered (via a DRAM bounce) to partition
    base 32*b so the K=1 rank-1 PV update satisfies tile_position rules.
- bk is never added: softmax is invariant to a per-query logit shift.
- Softmax skips max-subtraction; Z rides as a 65th ones-column of V; ctx is
  normalized by a DVE divide against a Z row broadcast across partitions
  with a DRAM-bounce stride-0 DMA (one bounce per head pair).
- Out-projection of batch 0 is interleaved into batch 1's attention loop.
"""

import sys

sys.path.insert(0, "/opt/trn_rl_repo")

import numpy as np
import ml_dtypes

import concourse.bass as bass
import concourse.mybir as mybir
import concourse.tile as tile
from concourse import bacc
from concourse.bass_utils import run_bass_kernel_spmd
from concourse.masks import make_identity

B, T, D = 16, 577, 1024
L, D_LLM = 64, 4096
H, HD = 16, 64
SCALE = HD ** -0.5
S = L + T            # 641 kv rows
NC = 8               # cores
BPC = B // NC        # batches per core = 2

F32 = mybir.dt.float32
BF16 = mybir.dt.bfloat16
Exp = mybir.ActivationFunctionType.Exp
Identity = mybir.ActivationFunctionType.Identity
MULT = mybir.AluOpType.mult


def _ap(base, offset_delta, dims):
    """Hand-built AP: keep base's tensor/partition dim, custom free dims."""
    return bass.AP(
        tensor=base.tensor,
        offset=base.offset + offset_delta,
        ap=[list(base.ap[0])] + [list(d) for d in dims],
    )


def build_nc(zero_bias: bool, debug: bool = False):
    nc = bacc.Bacc(None, target_bir_lowering=False)

    hskv_d = nc.dram_tensor("hskv", [128, 8, BPC, T], BF16, kind="ExternalInput")
    lrT_d = nc.dram_tensor("lrT", [128, 32, BPC * L], BF16, kind="ExternalInput")
    WqTj_d = nc.dram_tensor("WqTj", [128, 8, 8, 128], BF16, kind="ExternalInput")
    WkT_d = nc.dram_tensor("WkT", [128, 8, D], BF16, kind="ExternalInput")
    WvT_d = nc.dram_tensor("WvT", [128, 8, D], BF16, kind="ExternalInput")
    WoT_d = nc.dram_tensor("WoT", [128, 8, D], BF16, kind="ExternalInput")
    WpT_d = nc.dram_tensor("WpT", [128, 32, D], BF16, kind="ExternalInput")
    if not zero_bias:
        bq_d = nc.dram_tensor("bq2", [128, 8], F32, kind="ExternalInput")
        bp_d = nc.dram_tensor("bp2", [128, 8], F32, kind="ExternalInput")
        bv_d = nc.dram_tensor("bv2", [128, 8], F32, kind="ExternalInput")
        bo_d = nc.dram_tensor("bo2", [1, D], BF16, kind="ExternalInput")
    out_d = nc.dram_tensor("out2", [BPC, T, D], BF16, kind="ExternalOutput")
    if debug:
        dbg = {
            "dQT": nc.dram_tensor("dQT", [128, 8, BPC, T], BF16,
                                  kind="ExternalOutput"),
            "dKT": nc.dram_tensor("dKT", [128, 8, BPC, S], BF16,
                                  kind="ExternalOutput"),
            "dV": nc.dram_tensor("dV", [128, 5, BPC, H * 65], BF16,
                                 kind="ExternalOutput"),
            "dkv": nc.dram_tensor("dkv", [128, 8, BPC, S], BF16,
                                  kind="ExternalOutput"),
            "des4": nc.dram_tensor("des4", [64, 16, 640], BF16,
                                   kind="ExternalOutput"),
            "dvstrag": nc.dram_tensor("dvstrag", [64, 16, 65], BF16,
                                      kind="ExternalOutput"),
            "dkstragT": nc.dram_tensor("dkstragT", [128, 8, BPC, 16], BF16,
                                       kind="ExternalOutput"),
            "dctxT": nc.dram_tensor("dctxT", [128, 8, BPC, T], BF16,
                                    kind="ExternalOutput"),
        }

    with tile.TileContext(nc) as tc:
        with (
            tc.tile_pool(name="big", bufs=1) as big,
            tc.tile_pool(name="esp", bufs=2) as esp,
            tc.tile_pool(name="drp", bufs=4, space="DRAM") as drp,
        ):
            QT = big.tile([128, 8, BPC, T], BF16, tag="qt")
            KT = big.tile([128, 8, BPC, S], BF16, tag="kt")
            V = big.tile([128, 5, BPC, H * 65], BF16, tag="v")
            ctxT = big.tile([128, 8, BPC, T], BF16, tag="ctx")
            vstrag = big.tile([64, 16, 65], BF16, tag="vstrag")
            kstragT = big.tile([128, 8, BPC, 16], BF16, tag="kstragT")
            es4 = big.tile([64, 16, 640], BF16, tag="es4")
            if not zero_bias:
                bq_sb = big.tile([128, 8], F32, tag="bq")
                bp_sb = big.tile([128, 8], F32, tag="bp")
                bv_sb = big.tile([128, 8], F32, tag="bv")
                bo_sb = big.tile([1, D], BF16, tag="bo")
                ones1 = big.tile([1, 128], BF16, tag="ones1")

            def evac(dst, src, scalar_eng, bias=None):
                if bias is not None:
                    nc.scalar.activation(dst, src, Identity, bias=bias)
                elif scalar_eng:
                    nc.scalar.copy(dst, src)
                else:
                    nc.vector.tensor_copy(dst, src)

            with (
                tc.tile_pool(name="kvpool", bufs=1) as kvpool,
                tc.tile_pool(name="wpool", bufs=2) as wpool,
                tc.tile_pool(name="wppool", bufs=2) as wppool,
                tc.tile_pool(name="pp", bufs=2, space="PSUM") as pp,
                tc.tile_pool(name="latp_p", bufs=1, space="PSUM") as latp_p,
            ):
                kv = kvpool.tile([128, 8, BPC, S], BF16, tag="kv")
                if debug:
                    nc.vector.memset(vstrag, 0.0)
                    nc.vector.memset(es4, 0.0)
                    nc.vector.memset(kv, 0.0)
                lr_sb = kvpool.tile([128, 32, BPC * L], BF16, tag="lr")

                # ---- DMA kickoff: few chunky DMAs (HWDGE issue ~0.6us) ----
                wq = wpool.tile([128, 8, 8, 128], BF16, tag="w", name="wq")
                nc.sync.dma_start(wq[:, 0], WqTj_d[:, 0])
                nc.sync.dma_start(kv[:, 0:2, 0, L:S], hskv_d[:, 0:2, 0, :])
                nc.sync.dma_start(kv[:, 2:8, 0, L:S], hskv_d[:, 2:8, 0, :])
                nc.sync.dma_start(wq[:, 1:4], WqTj_d[:, 1:4])
                for k in range(0, 8, 4):
                    nc.sync.dma_start(kv[:, k:k + 4, 1, L:S],
                                      hskv_d[:, k:k + 4, 1, :])
                nc.sync.dma_start(wq[:, 4:8], WqTj_d[:, 4:8])
                wk = wpool.tile([128, 8, D], BF16, tag="w", name="wk")
                for i in range(2):
                    nc.sync.dma_start(wk[:, 4 * i:4 * i + 4, :],
                                      WkT_d[:, 4 * i:4 * i + 4, :])
                nc.sync.dma_start(lr_sb, lrT_d[:, :, :])
                if not zero_bias:
                    nc.sync.dma_start(bq_sb, bq_d[:, :])
                    nc.sync.dma_start(bp_sb, bp_d[:, :])
                    nc.sync.dma_start(bv_sb, bv_d[:, :])
                    nc.sync.dma_start(bo_sb, bo_d[:, :])
                    nc.vector.memset(ones1, 1.0)

                # ---- Q projection: QT[o, t] ----
                for b in range(BPC):
                    for j in range(8):
                        ps = pp.tile([128, 2, 512], F32, tag="pq")
                        for k in range(8):
                            nc.tensor.matmul(
                                ps[:, 0, :], wq[:, j, k, :],
                                kv[:, k, b, L:L + 512],
                                start=(k == 0), stop=(k == 7))
                        for k in range(8):
                            nc.tensor.matmul(
                                ps[:, 1, 0:65], wq[:, j, k, :],
                                kv[:, k, b, L + 512:S],
                                start=(k == 0), stop=(k == 7))
                        bias = None if zero_bias else bq_sb[:, j:j + 1]
                        evac(QT[:, j, b, 0:512], ps[:, 0, :], True, bias)
                        evac(QT[:, j, b, 512:T], ps[:, 1, 0:65], False, bias)

                # wv reuses wq's buffer (Q-proj consumed it above)
                wv = wpool.tile([128, 8, D], BF16, tag="w", name="wv")
                for i in range(4):
                    nc.sync.dma_start(wv[:, 2 * i:2 * i + 2, :],
                                      WvT_d[:, 2 * i:2 * i + 2, :])

                def k_hi(b):
                    """K projection, tokens part (s = 64..641); no bk ever."""
                    for j in range(8):
                        ps = pp.tile([128, 2, 512], F32, tag="pq", name="psk")
                        for k in range(8):
                            nc.tensor.matmul(
                                ps[:, 0, :], wk[:, k, j * 128:(j + 1) * 128],
                                kv[:, k, b, L:L + 512],
                                start=(k == 0), stop=(k == 7))
                        for k in range(8):
                            nc.tensor.matmul(
                                ps[:, 1, 0:65], wk[:, k, j * 128:(j + 1) * 128],
                                kv[:, k, b, L + 512:S],
                                start=(k == 0), stop=(k == 7))
                        evac(KT[:, j, b, L:L + 512], ps[:, 0, :], True)
                        evac(KT[:, j, b, L + 512:S], ps[:, 1, 0:65], False)

                ident = kvpool.tile([128, 128], BF16, tag="ident")
                make_identity(nc, ident)
                k_hi(0)

                # ---- straggler row s=640 (token 576): K and V, transposed ----
                ps = pp.tile([128, 2, 512], F32, tag="pq", name="ps8")
                s8p = ps[:, 0, 0:32].rearrange("p (o c) -> p o c", c=4)
                for oc in range(8):
                    for k in range(8):
                        nc.tensor.matmul(
                            s8p[:, oc, 0:2], wk[:, k, oc * 128:(oc + 1) * 128],
                            kv[:, k, 0:BPC, S - 1],
                            start=(k == 0), stop=(k == 7))
                    for k in range(8):
                        nc.tensor.matmul(
                            s8p[:, oc, 2:4], wv[:, k, oc * 128:(oc + 1) * 128],
                            kv[:, k, 0:BPC, S - 1],
                            start=(k == 0), stop=(k == 7))
                s8 = kvpool.tile([128, 8, 4], BF16, tag="s8")
                nc.vector.tensor_copy(s8, s8p)

                # kstragT[p, k, b, h] = K[128k+p, 640] iff head(128k+p)==h
                nc.vector.memset(kstragT, 0.0)
                s8f = s8.rearrange("p o c -> p (o c)")
                kf = kstragT.rearrange("p k b h -> p (k b h)")
                for half in range(2):
                    pr = slice(64 * half, 64 * half + 64)
                    dst = _ap(kf[pr, 0:1], half, [[34, 8], [16, BPC]])
                    src = _ap(s8f[pr, 0:1], 0, [[4, 8], [1, BPC]])
                    nc.vector.tensor_copy(dst, src)

                # vstrag slot-scatter via DRAM bounce: slot (h, b) lives at
                # partition 32*b, free slot h
                d8 = drp.tile([128, 8, 4], BF16, tag="d8")
                nc.gpsimd.dma_start(d8, s8)
                nc.vector.memset(vstrag[:, :, 64:65], 1.0)
                for b in range(BPC):
                    for h in range(16):
                        par, oc = h % 2, h // 2
                        dst = vstrag[32 * b:32 * b + 1, h, 0:64]
                        src = _ap(d8[0:1, 0:1, 0],
                                  64 * par * 32 + oc * 4 + 2 + b, [[32, 64]])
                        nc.gpsimd.dma_start(dst, src)

                def strag_logits(b):
                    """All 16 heads' straggler logits -> es4, computed
                    t-partitioned (N=16 matmuls), then PE-transposed."""
                    plt = pp.tile([128, 2, 512], F32, tag="pq", name="pls")
                    for tcn in range(5):
                        t0 = tcn * 128
                        m = min(128, T - t0)
                        for k in range(8):
                            nc.tensor.matmul(
                                plt[0:m, 0, tcn * 16:(tcn + 1) * 16],
                                QT[:, k, b, t0:t0 + m],
                                kstragT[:, k, b, :],
                                start=(k == 0), stop=(k == 7))
                    es_t = esp.tile([128, 80], BF16, tag="est")
                    nc.scalar.activation(es_t[:, 0:64], plt[:, 0, 0:64], Exp,
                                         bias=0.0, scale=SCALE)
                    nc.scalar.activation(es_t[0:65, 64:80],
                                         plt[0:65, 0, 64:80], Exp,
                                         bias=0.0, scale=SCALE)
                    pt = latp_p.tile([16, 5, 128], BF16, tag="pt")
                    for tcn in range(5):
                        m = min(128, T - tcn * 128)
                        nc.tensor.transpose(pt[:, tcn, 0:m],
                                            es_t[0:m, tcn * 16:(tcn + 1) * 16],
                                            ident[0:m, 0:m])
                    es_sb = esp.tile([16, 640], BF16, tag="es")
                    nc.vector.memset(es_sb[:, T:640], 0.0)
                    ptf = pt.rearrange("p g c -> p (g c)")
                    nc.vector.tensor_copy(es_sb[:, 0:T], ptf[:, 0:T])
                    esd = drp.tile([16, 640], BF16, tag="esd")
                    nc.gpsimd.dma_start(esd, es_sb)
                    dst = es4[32 * b:32 * b + 1, :, :].rearrange(
                        "p f c -> p (f c)")
                    src = _ap(esd[0:1, 0:1], 0, [[1, 16 * 640]])
                    nc.gpsimd.dma_start(dst, src)

                strag_logits(0)
                k_hi(1)
                strag_logits(1)

                # ---- V proj main (s-chunks 1..4) interleaved with latt ----
                # latt in [lat, o] orientation: one accumulation group per
                # psum bank (a start= claims a whole 2KB zero region)
                latp = latp_p.tile([128, 2, 512], F32, tag="lat")

                def v_main(b, scs):
                    for sc in scs:
                        ps = pp.tile([128, 2, 512], F32, tag="pq", name="psv")
                        for oh in range(2):
                            for k in range(8):
                                nc.tensor.matmul(
                                    ps[:, oh, :],
                                    kv[:, k, b, sc * 128:(sc + 1) * 128],
                                    wv[:, k, oh * 512:(oh + 1) * 512],
                                    start=(k == 0), stop=(k == 7))
                        vv = V[:, sc, b, :].rearrange("p (h c) -> p h c", c=65)
                        for oh in range(2):
                            evac(vv[:, oh * 8:(oh + 1) * 8, 0:64],
                                 ps[:, oh, :].rearrange(
                                     "p (h c) -> p h c", c=64),
                                 oh == 0)

                def latt_q(hq):
                    """One eighth (4 k-chunks) of the latt contraction."""
                    wp = wppool.tile([128, 4, D], BF16, tag="wp")
                    nc.sync.dma_start(wp, WpT_d[:, hq * 4:(hq + 1) * 4, :])
                    for k in range(4):
                        for oh in range(2):
                            nc.tensor.matmul(
                                latp[:, oh, :],
                                lr_sb[:, hq * 4 + k, :],
                                wp[:, k, oh * 512:(oh + 1) * 512],
                                start=(hq == 0 and k == 0),
                                stop=(hq == 7 and k == 3))

                v_main(0, (1, 2))
                latt_q(0)
                latt_q(1)
                v_main(0, (3, 4))
                latt_q(2)
                latt_q(3)
                v_main(1, (1, 2))
                latt_q(4)
                latt_q(5)
                v_main(1, (3, 4))
                latt_q(6)
                latt_q(7)
                latn = kvpool.tile([128, D], BF16, tag="latn")
                evac(latn[:, 0:512], latp[:, 0, :], True)
                evac(latn[:, 512:D], latp[:, 1, :], False)
                for oc in range(8):
                    ptb = latp_p.tile([128, 128], BF16, tag="ptb")
                    nc.tensor.transpose(
                        ptb, latn[:, oc * 128:(oc + 1) * 128], ident)
                    bias = None if zero_bias else bp_sb[:, oc:oc + 1]
                    evac(kv[:, oc, :, 0:L],
                         ptb[:, :].rearrange("p (b l) -> p b l", b=BPC),
                         oc % 2 == 0, bias)

                # ---- K proj latents part (s 0..64) + V chunk 0, mixed ----
                def k_lo(j):
                    ps = pp.tile([128, 2, 512], F32, tag="pq", name="pkl")
                    for b in range(BPC):
                        for k in range(8):
                            nc.tensor.matmul(
                                ps[:, b, 0:L],
                                wk[:, k, j * 128:(j + 1) * 128],
                                kv[:, k, b, 0:L],
                                start=(k == 0), stop=(k == 7))
                    evac(KT[:, j, :, 0:L],
                         ps[:, 0:2, 0:L], j % 2 == 0)

                def v_0(b):
                    ps = pp.tile([128, 2, 512], F32, tag="pq", name="psv0")
                    for oh in range(2):
                        for k in range(8):
                            nc.tensor.matmul(
                                ps[:, oh, :], kv[:, k, b, 0:128],
                                wv[:, k, oh * 512:(oh + 1) * 512],
                                start=(k == 0), stop=(k == 7))
                    vv = V[:, 0, b, :].rearrange("p (h c) -> p h c", c=65)
                    for oh in range(2):
                        evac(vv[:, oh * 8:(oh + 1) * 8, 0:64],
                             ps[:, oh, :].rearrange("p (h c) -> p h c", c=64),
                             oh == 0)

                k_lo(0)
                k_lo(1)
                v_0(0)
                k_lo(2)
                k_lo(3)
                k_lo(4)
                v_0(1)
                k_lo(5)
                k_lo(6)
                k_lo(7)
                if debug:
                    nc.sync.dma_start(dbg["dkv"][:, :, :, :], kv)
                # ones column of V (all rows 0..640 are valid keys)
                vva = V[:, :, :, :].rearrange("p s b (h c) -> p s b h c", c=65)
                nc.vector.memset(vva[:, :, :, :, 64:65], 1.0)

            # ---- attention + out-projection (PV software-pipelined) ----
            with (
                tc.tile_pool(name="att", bufs=1) as att,
                tc.tile_pool(name="eap", bufs=12) as eap,
                tc.tile_pool(name="ebp", bufs=3) as ebp,
                tc.tile_pool(name="pvsp", bufs=3) as pvsp,
                tc.tile_pool(name="zbp", bufs=3) as zbp,
                tc.tile_pool(name="zrp", bufs=3) as zrp,
                tc.tile_pool(name="osbp", bufs=3) as osbp,
                tc.tile_pool(name="pa", bufs=4, space="PSUM") as pa_p,
            ):
                wo = att.tile([128, 8, D], BF16, tag="wo")
                for i in range(2):
                    nc.sync.dma_start(wo[:, 4 * i:4 * i + 4, :],
                                      WoT_d[:, 4 * i:4 * i + 4, :])

                def out_proj(b, tcn, scalar_evac):
                    t0 = tcn * 128
                    m = min(128, T - t0)
                    po = pa_p.tile([128, 2, 512], F32, tag="pa", name="po")
                    osb = osbp.tile([128, D], BF16, tag="osb")
                    for oc in range(2):
                        for k in range(8):
                            nc.tensor.matmul(
                                po[:m, oc, :], ctxT[:, k, b, t0:t0 + m],
                                wo[:, k, oc * 512:(oc + 1) * 512],
                                start=(k == 0), stop=(zero_bias and k == 7))
                        if not zero_bias:
                            nc.tensor.matmul(
                                po[:m, oc, :], ones1[0:1, :m],
                                bo_sb[0:1, oc * 512:(oc + 1) * 512],
                                start=False, stop=True)
                        evac(osb[:m, oc * 512:(oc + 1) * 512], po[:m, oc, :],
                             scalar_evac == (oc == 0))
                        nc.sync.dma_start(
                            out_d[b, t0:t0 + m, oc * 512:(oc + 1) * 512],
                            osb[:m, oc * 512:(oc + 1) * 512])

                def lg(jp, b, ebv, ea, g):
                    """One logits s-chunk (t 0:512 part) + its exp."""
                    pa = pa_p.tile([128, 2, 512], F32, tag="pa")
                    for hh in range(2):
                        hb = 64 * hh
                        kt = KT[hb:hb + 64, jp, b, g * 128:(g + 1) * 128]
                        nc.tensor.matmul(
                            pa[:, hh, :], kt, QT[hb:hb + 64, jp, b, 0:512],
                            start=True, stop=True)
                    eg = eap.tile([128, 2, 512], BF16, tag="ea")
                    nc.scalar.activation(eg, pa, Exp, bias=0.0, scale=SCALE)
                    ea.append(eg)

                def eb_mms(jp, b, ebv):
                    """All 65-col logits matmuls (t 512:577) + one exp."""
                    for g in range(5):
                        for hh in range(2):
                            hb = 64 * hh
                            kt = KT[hb:hb + 64, jp, b, g * 128:(g + 1) * 128]
                            nc.tensor.matmul(
                                ebv[1][hh][:, g, :], kt,
                                QT[hb:hb + 64, jp, b, 512:T],
                                start=True, stop=True)
                    ebs = ebp.tile([128, 2, 5, 65], BF16, tag="eb")
                    nc.scalar.activation(
                        ebs, ebv[0][:, :, 0:325].rearrange(
                            "p h (g c) -> p h g c", c=65),
                        Exp, bias=0.0, scale=SCALE)
                    return ebs

                def pv_mm(st, sc):
                    (jp, b, ea, ebs, pvt) = st
                    hh = sc // 6
                    h = 2 * jp + hh
                    pv0 = pvt[hh][0:65, 0, :]
                    pv1 = pvt[hh][0:65, 1, 0:65]
                    scc = sc % 6
                    if scc < 5:
                        vh = V[:, scc, b, 65 * h:65 * h + 65]
                        nc.tensor.matmul(pv0, vh, ea[scc][:, hh, :],
                                         start=(scc == 0), stop=False)
                        nc.tensor.matmul(pv1, vh, ebs[:, hh, scc, :],
                                         start=(scc == 0), stop=False)
                    else:
                        nc.tensor.matmul(
                            pv0, vstrag[32 * b:32 * b + 1, h, :],
                            es4[32 * b:32 * b + 1, h, 0:512],
                            start=False, stop=True)
                        nc.tensor.matmul(
                            pv1, vstrag[32 * b:32 * b + 1, h, :],
                            es4[32 * b:32 * b + 1, h, 512:T],
                            start=False, stop=True)

                def pv_evac(st, pvs, zr, hh):
                    pvf = st[4][hh].rearrange("p a c -> p (a c)")
                    nc.vector.tensor_copy(pvs[0:64, hh, 0:T], pvf[0:64, 0:T])
                    nc.vector.reciprocal(zr[:, hh, 0:T], pvf[64:65, 0:T])

                def z_bounce(st, pvs, zr):
                    (jp, b) = st[0], st[1]
                    zd = drp.tile([1, 2, T], F32, tag="zd")
                    nc.sync.dma_start(zd, zr[0:1, :, 0:T])
                    zb = zbp.tile([64, 2, T], F32, tag="zb")
                    for hh in range(2):
                        zsrc = zd[0:1, hh, :]
                        src = bass.AP(
                            tensor=zsrc.tensor, offset=zsrc.offset,
                            ap=[[0, 64]] + [list(d) for d in zsrc.ap[1:]])
                        nc.gpsimd.dma_start(zb[:, hh, :], src)

                    def divides():
                        for hh in range(2):
                            hb = 64 * hh
                            nc.vector.tensor_tensor(
                                ctxT[hb:hb + 64, jp, b, 0:T],
                                pvs[0:64, hh, 0:T],
                                zb[:, hh, 0:T], MULT)
                            if not zero_bias:
                                nc.vector.tensor_scalar_add(
                                    ctxT[hb:hb + 64, jp, b, 0:T],
                                    ctxT[hb:hb + 64, jp, b, 0:T],
                                    bv_sb[hb:hb + 64, jp:jp + 1])
                    return divides

                iters = [(jp, b) for b in range(BPC) for jp in range(8)]
                prev = None      # (jp, b, ea, ebs, pvt) awaiting PV
                pending = []     # divide closures awaiting flush
                for it in iters + [None]:
                    if len(pending) >= 2:
                        pending.pop(0)()
                    if prev is not None:
                        pvs = pvsp.tile([64, 2, 578], F32, tag="pvs")
                        zr = zrp.tile([1, 2, 578], F32, tag="zr")
                    if it is not None:
                        (jp, b) = it
                        ea = []
                        ebt = pa_p.tile([128, 2, 512], F32, tag="pa",
                                        name="ebt")
                        ebv = (ebt, [ebt[:, hh, 0:325].rearrange(
                            "p (g c) -> p g c", c=65) for hh in range(2)])
                        lg(jp, b, ebv, ea, 0)
                        if prev is not None:
                            prev[4].append(pa_p.tile(
                                [128, 2, 512], F32, tag="pa", name="pvt0"))
                            pv_mm(prev, 0)
                            pv_mm(prev, 1)
                        lg(jp, b, ebv, ea, 1)
                        lg(jp, b, ebv, ea, 2)
                        if prev is not None:
                            pv_mm(prev, 2)
                            pv_mm(prev, 3)
                        lg(jp, b, ebv, ea, 3)
                        if prev is not None:
                            pv_mm(prev, 4)
                            pv_mm(prev, 5)
                            pv_evac(prev, pvs, zr, 0)
                            prev[4].append(pa_p.tile(
                                [128, 2, 512], F32, tag="pa", name="pvt1"))
                            for sc in (6, 7, 8):
                                pv_mm(prev, sc)
                        lg(jp, b, ebv, ea, 4)
                        if prev is not None:
                            for sc in (9, 10, 11):
                                pv_mm(prev, sc)
                            pv_evac(prev, pvs, zr, 1)
                            pending.append(z_bounce(prev, pvs, zr))
                        ebs = eb_mms(jp, b, ebv)
                        if b == 1 and 2 <= jp <= 6:
                            out_proj(0, jp - 2, False)
                        prev = (jp, b, ea, ebs, [])
                    else:
                        # drain: PV of the last iteration, interleaved with
                        # the k0..k6 accumulation of the first two tail OPs
                        # (their k=7 chunk needs the last divide)
                        for p_ in pending:
                            p_()

                        def op_mms(tcn, po, ks):
                            t0 = tcn * 128
                            m = min(128, T - t0)
                            for oc in range(2):
                                for k in ks:
                                    nc.tensor.matmul(
                                        po[:m, oc, :],
                                        ctxT[:, k, 1, t0:t0 + m],
                                        wo[:, k, oc * 512:(oc + 1) * 512],
                                        start=(k == 0),
                                        stop=(zero_bias and k == 7))

                        def op_fin(tcn, po):
                            t0 = tcn * 128
                            m = min(128, T - t0)
                            if not zero_bias:
                                for oc in range(2):
                                    nc.tensor.matmul(
                                        po[:m, oc, :], ones1[0:1, :m],
                                        bo_sb[0:1, oc * 512:(oc + 1) * 512],
                                        start=False, stop=True)
                            osb = osbp.tile([128, D], BF16, tag="osb")
                            for oc in range(2):
                                evac(osb[:m, oc * 512:(oc + 1) * 512],
                                     po[:m, oc, :], oc == 0)
                                nc.sync.dma_start(
                                    out_d[1, t0:t0 + m,
                                          oc * 512:(oc + 1) * 512],
                                    osb[:m, oc * 512:(oc + 1) * 512])

                        prev[4].append(pa_p.tile(
                            [128, 2, 512], F32, tag="pa", name="pvt0"))
                        po_t0 = pa_p.tile([128, 2, 512], F32, tag="pa",
                                          name="po_t0")
                        op_mms(0, po_t0, range(7))
                        for sc in range(6):
                            pv_mm(prev, sc)
                        pv_evac(prev, pvs, zr, 0)
                        prev[4].append(pa_p.tile(
                            [128, 2, 512], F32, tag="pa", name="pvt1"))
                        po_t1 = pa_p.tile([128, 2, 512], F32, tag="pa",
                                          name="po_t1")
                        op_mms(1, po_t1, range(7))
                        for sc in range(6, 12):
                            pv_mm(prev, sc)
                        pv_evac(prev, pvs, zr, 1)
                        z_bounce(prev, pvs, zr)()
                        op_mms(0, po_t0, range(7, 8))
                        op_fin(0, po_t0)
                        op_mms(1, po_t1, range(7, 8))
                        op_fin(1, po_t1)

                for tcn in range(2, 5):
                    out_proj(1, tcn, True)
                if debug:
                    nc.sync.dma_start(dbg["dQT"][:, :, :], QT)
                    nc.sync.dma_start(dbg["dKT"][:, :, :], KT)
                    nc.sync.dma_start(dbg["dV"][:, :, :], V)
                    nc.sync.dma_start(dbg["des4"][:, :, :], es4)
                    nc.sync.dma_start(dbg["dvstrag"][:, :, :], vstrag)
                    nc.sync.dma_start(dbg["dkstragT"][:, :, :], kstragT)
                    nc.sync.dma_start(dbg["dctxT"][:, :, :], ctxT)

    nc.finalize()
    return nc


_NC_CACHE = {}
LAST_RESULT = None


def kernel(hidden_states, latt_raw, Wp, bp, Wq, bq, Wk, bk, Wv, bv, Wo, bo,
           trace=False):
    global LAST_RESULT
    f = lambda x: np.ascontiguousarray(np.asarray(x), dtype=np.float32)
    hs, lr = f(hidden_states), f(latt_raw)
    Wp, Wq, Wk, Wv, Wo = f(Wp), f(Wq), f(Wk), f(Wv), f(Wo)
    bp, bq, bk, bv, bo = f(bp), f(bq), f(bk), f(bv), f(bo)

    # bk never matters: softmax is invariant to the per-query shift q . bk
    zero_bias = not any(x.any() for x in (bp, bq, bv, bo))
    key = zero_bias
    if key not in _NC_CACHE:
        _NC_CACHE[key] = build_nc(zero_bias)
    nc = _NC_CACHE[key]

    bf = ml_dtypes.bfloat16
    # [j, p, k, m] = Wq[128j+m, 128k+p]
    WqTj = np.ascontiguousarray(
        Wq.reshape(8, 128, 8, 128).transpose(3, 0, 2, 1).astype(bf))
    # [p, k, o] = W[o, 128k+p]
    t8 = lambda W: np.ascontiguousarray(
        W.T.reshape(8, 128, D).transpose(1, 0, 2).astype(bf))
    WkT, WvT, WoT = t8(Wk), t8(Wv), t8(Wo)
    WpT = np.ascontiguousarray(
        Wp.T.reshape(32, 128, D).transpose(1, 0, 2).astype(bf))

    b2 = lambda x: np.ascontiguousarray(x.reshape(8, 128).T)
    bias_maps = {}
    if not zero_bias:
        bias_maps = {
            "bq2": b2(bq), "bp2": b2(bp), "bv2": b2(bv),
            "bo2": np.ascontiguousarray(bo[None, :].astype(bf)),
        }

    in_maps = []
    for c in range(NC):
        hsb = hs[c * BPC:(c + 1) * BPC]              # [2, 577, 1024]
        hskv = np.ascontiguousarray(
            hsb.reshape(BPC, T, 8, 128).transpose(3, 2, 0, 1).astype(bf))
        lrb = lr[c * BPC:(c + 1) * BPC]              # [2, 64, 4096]
        lrT = np.ascontiguousarray(
            lrb.reshape(BPC, L, 32, 128).transpose(3, 2, 0, 1)
            .reshape(128, 32, BPC * L).astype(bf))
        m = {"hskv": hskv, "lrT": lrT, "WqTj": WqTj, "WkT": WkT,
             "WvT": WvT, "WoT": WoT, "WpT": WpT}
        m.update(bias_maps)
        in_maps.append(m)

    LAST_RESULT = run_bass_kernel_spmd(
        nc, in_maps, core_ids=list(range(NC)), trace=trace
    )
    outs = [r["out2"] for r in LAST_RESULT.results]
    return np.ascontiguousarray(np.concatenate(outs, axis=0), dtype=np.float32)
